# revision 61
# baseline (speedup 1.0000x reference)
"""Trainium2 Bass kernel for nn_MultiHeadAttention (B=4,T=1024,C=1024,H=16).

Sharding: 8 cores = 4 batches x 2 query-halves. Each core computes, for its
batch b and its 512 query rows:
  V projection (natural layout, mask folded in, +mask column for denominator),
  then per head-pair: Q^T/K^T projection chunks, S^T = K^T.T @ Q^T (row-tiled
  head pairs, D=64 contraction), one exp ACT per key chunk over the merged
  two-head S psum, O^T+denominator via one augmented matmul lhsT=[V_h*m | m],
  normalize via reciprocal + DRAM-bounce partition-broadcast; finally
  Y = O^T.T @ Wo with LN stats read straight from PSUM and the LN apply on
  ACT. Keys are mask-sorted host-side so masked tail chunks drop (NK=5 of 8).

Perf notes (HW ~152us baseline -> ~140-142us this version, cool chip;
sustained back-to-back runs can thermally throttle the PE to ~165us):
  - PE HAM clock gate: the PE runs at 1.2GHz until ~3.4us of sustained
    activity and re-throttles after idle windows, so idle gaps cost double.
    NWARM warmup matmuls on a zeroed tile hold the clock gate open while
    the first input DMAs stream (first real matmul ~8us vs ~16us before).
  - each pair's O^T matmuls are emitted as the NEXT pair's fill work
    (after its QK fill): they depend only on already-finished exps, so
    they are always-ready cover for the serial per-chunk exp chain.
  - Wo-projection chain qc0 (on the freed psP bufs) is emitted as pair 7's
    fill work; qc1 (po bufs), qc2 (s01 bufs), qc3 (po bufs again) follow
    right after the pair loop, each on disjoint psum banks so no chain
    waits on another's LN apply; all four finishers run at the end where
    pair 7's normalize has certainly drained.
  - only sync+scalar have fast (~105 GB/s each) HW DMA queues; V-projection
    inputs land kc-interleaved across both queues in first-use order so
    the first chains trickle along with the DMA instead of stalling on a
    full half; the remaining tensors stream as chunk-halves.
  - fp8/DoubleRow paths exist behind BASS_QK_FP8/BASS_WO_FP8 but are OFF:
    e4m3 quantization injects ~2-4% relative noise into every dot product
    (random-sign sums keep per-element relative error), busting the 2e-2
    gate (measured 4.3e-2 with fp8 projections).
Host gathers the 8 [512,1024] outputs into [4,1024,1024].
"""

import os
import sys

import numpy as np

for _p in ("/opt/trn_rl_repo", "/root/.axon_site/_ro/trn_rl_repo"):
    if os.path.isdir(_p) and _p not in sys.path:
        sys.path.append(_p)

import ml_dtypes  # noqa: E402
import concourse.bass as bass  # noqa: E402
import concourse.mybir as mybir  # noqa: E402
import concourse.tile as tile  # noqa: E402
from concourse import bacc  # noqa: E402
from concourse.bass_utils import run_bass_kernel_spmd  # noqa: E402

BF16 = mybir.dt.bfloat16
FP8 = mybir.dt.float8e4
F32 = mybir.dt.float32
NPBF16 = ml_dtypes.bfloat16
NPFP8 = ml_dtypes.float8_e4m3fn
DR = mybir.MatmulPerfMode.DoubleRow
WSCALE = 64.0     # weights stored x64 in fp8 (keeps them out of subnormals)
WO_FP8 = os.environ.get("BASS_WO_FP8", "0") == "1"
QK_FP8 = os.environ.get("BASS_QK_FP8", "0") == "1"
PROJ_T = FP8 if QK_FP8 else BF16
PSCALE = WSCALE if QK_FP8 else 1.0
OSCALE = 16.0 if WO_FP8 else 1.0  # normalized O^T stored x16 in fp8 (LN absorbs it)
OT_T = FP8 if WO_FP8 else BF16

B, T, C, H = 4, 1024, 1024, 16
D = C // H          # 64
P = 128             # partitions
NC = C // P         # 8 chunks of C
NT = T // P         # 8 chunks of T
TQ = T // 2         # 512 query rows per core
NQ = TQ // P        # 4 query chunks
NPAIR = H // 2      # 8 head pairs
EPS = 1e-5

_CACHE = {}
LAST_RESULTS = None


def _ensure_ntff_hook():
    """Register the axon NTFF profiling hook if the image's antenv lacks it."""
    try:
        import antenv.axon_hooks  # noqa: F401
        return
    except ImportError:
        pass
    try:
        import types

        import antenv
        from trn_agent_boot.trn_boot import _ntff_profile_via_ctypes

        mod = types.ModuleType("antenv.axon_hooks")
        state = {"hook": None}
        mod.set_axon_ntff_profile_hook = lambda h: state.__setitem__("hook", h)
        mod.get_axon_ntff_profile_hook = lambda: state["hook"]
        sys.modules["antenv.axon_hooks"] = mod
        antenv.axon_hooks = mod
        hook = _ntff_profile_via_ctypes("/opt/axon/libaxon_pjrt.so")
        if hook is not None:
            mod.set_axon_ntff_profile_hook(hook)
    except Exception:
        pass


def _emit(nc, tc, dr, NK, use_lnw, use_boe):
    """Emit the per-core Tile program (projections interleaved with attention)."""
    from contextlib import ExitStack

    AF = mybir.ActivationFunctionType
    OP = mybir.AluOpType
    NWARM = int(os.environ.get("BASS_NWARM", "12"))

    with ExitStack() as ctx:
        consts = ctx.enter_context(tc.tile_pool(name="consts", bufs=1))

        # ---- persistent SBUF tiles ----
        KL = NK * P
        VA = [
            consts.tile([P, H, D + 1], BF16, tag=f"va{j}", name=f"va{j}")
            for j in range(NK)
        ]
        # OT pair tiles: OTp[j][:, m, :] = head-pair 2j+m's normalized O^T
        OTp = [
            consts.tile([P, 2, TQ], OT_T, tag=f"otp{j}", name=f"otp{j}")
            for j in range(NPAIR // 2)
        ]
        Wo_sb = consts.tile([P, NC, C], OT_T)
        vecs = consts.tile([P, NC, 3], F32)        # bq | bk | maskf
        maskv = consts.tile([P, NK], BF16)
        eps_t = consts.tile([P, 1], F32)
        warm = consts.tile([P, TQ], BF16)
        if use_lnw:
            lng_rep = consts.tile([P, C], F32)
            lnb_rep = consts.tile([P, C], F32)
        if use_boe:
            boe_sb = consts.tile([1, C], BF16)     # bv@Wo+bo row (partition 0)
            ones_sb = consts.tile([1, P], BF16)    # ones row for bias preload
            nc.vector.memset(ones_sb, 1.0)

        nc.vector.memset(eps_t, EPS)
        nc.vector.memset(warm, 0.0)

        with (
            tc.tile_pool(name="pa", bufs=1) as pa,
            tc.tile_pool(name="pb", bufs=2) as pb,
            tc.tile_pool(name="pbd", bufs=2, space="DRAM") as pbd,
            tc.tile_pool(name="psP", bufs=2, space="PSUM") as psP,
            tc.tile_pool(name="psS", bufs=2, space="PSUM") as psS,
            tc.tile_pool(name="psO", bufs=2, space="PSUM") as psO,
        ):
            # ---- PE warmup: hold the HAM clock gate open while the input
            # DMAs stream; results are never read.
            wps = psS.tile([P, 2, TQ], F32, tag="s01", bufs=1, name="warmps")
            for _ in range(NWARM):
                nc.tensor.matmul(
                    wps[:, 0, :], warm[:, 0:P], warm[:], start=True, stop=True
                )

            xT = pa.tile([P, NC, KL], PROJ_T)
            xTq = pa.tile([P, NC, TQ], PROJ_T)
            Wq_sb = pa.tile([P, NC, C], PROJ_T)
            Wk_sb = pa.tile([P, NC, C], PROJ_T)
            Wv_sb = pa.tile([P, 2, NC, TQ], PROJ_T)  # nn-major

            # ---- input DMAs. Only sync and scalar have fast HW queues;
            # gpsimd's software queue only carries the small vectors. Each
            # tensor is split into two chunk-halves, one per queue, ordered
            # by first use: V-projection inputs first, Wo last.
            HNC = NC // 2
            xT_a = dr["xT"].ap()
            wv0_a = dr["Wv"].ap()[:, 0]
            wv1_a = dr["Wv"].ap()[:, 1]
            # V-projection inputs land kc-interleaved so the first chains
            # trickle along with the DMA instead of stalling on a full half.
            for eng, lo in ((nc.sync, 0), (nc.scalar, 4)):
                eng.dma_start(out=xT[:, lo : lo + 2], in_=xT_a[:, lo : lo + 2])
                eng.dma_start(
                    out=Wv_sb[:, 0, lo : lo + 2], in_=wv0_a[:, lo : lo + 2]
                )
                eng.dma_start(out=xT[:, lo + 2 : lo + 4], in_=xT_a[:, lo + 2 : lo + 4])
                eng.dma_start(
                    out=Wv_sb[:, 0, lo + 2 : lo + 4], in_=wv0_a[:, lo + 2 : lo + 4]
                )
                eng.dma_start(
                    out=Wv_sb[:, 1, lo : lo + 4], in_=wv1_a[:, lo : lo + 4]
                )
            halves = [
                ("xTq", xTq, xTq),
                ("Wq", Wq_sb, Wq_sb), ("Wk", Wk_sb, Wk_sb),
            ]
            for name, tl, _ in halves:
                a = dr[name].ap()
                nc.sync.dma_start(out=tl[:, 0:HNC], in_=a[:, 0:HNC])
                nc.scalar.dma_start(out=tl[:, HNC:], in_=a[:, HNC:])
            # Wo isn't consumed until the Wo chains (~95us in): ship it on
            # gpsimd's slow software queue (~40GB/s, lands ~60us) to free
            # the fast queues for the head-critical V/Q/K inputs.
            nc.gpsimd.dma_start(out=Wo_sb[:], in_=dr["Wo"].ap()[:])
            nc.gpsimd.dma_start(out=vecs[:], in_=dr["vecs"].ap()[:])
            nc.gpsimd.dma_start(out=maskv[:], in_=dr["maskv"].ap()[:])
            if use_boe:
                nc.gpsimd.dma_start(out=boe_sb[:], in_=dr["boe"].ap()[:])
            if use_lnw:
                for name, rep in (("lng", lng_rep), ("lnb", lnb_rep)):
                    a = dr[name].ap()
                    bcast = bass.AP(
                        tensor=a.tensor, offset=a.offset, ap=[[0, P], [1, C]]
                    )
                    nc.gpsimd.dma_start(out=rep[:], in_=bcast)

            # ---- V projection: natural [KL, C], masked rows, + mask col ----
            # nn-outer so the first 5 groups only need Wv's nn=0 columns.
            for nn in range(2):
                for tcn in range(NK):
                    ps = psP.tile([P, TQ], F32, tag="psp")
                    if QK_FP8:
                        for kc in range(0, NC, 2):
                            nc.tensor.matmul(
                                ps[:],
                                xT[:, kc : kc + 2, tcn * P : (tcn + 1) * P],
                                Wv_sb[:, nn, kc : kc + 2, :],
                                start=(kc == 0),
                                stop=(kc == NC - 2),
                                perf_mode=DR,
                            )
                    else:
                        for kc in range(NC):
                            nc.tensor.matmul(
                                ps[:],
                                xT[:, kc, tcn * P : (tcn + 1) * P],
                                Wv_sb[:, nn, kc, :],
                                start=(kc == 0),
                                stop=(kc == NC - 1),
                            )
                    nc.vector.tensor_scalar_mul(
                        VA[tcn][:, nn * 8 : (nn + 1) * 8, 0:D],
                        ps[:].rearrange("p (h d) -> p h d", h=8),
                        vecs[:, tcn, 2:3],
                    )
            for tcn in range(NK):
                nc.vector.tensor_copy(
                    out=VA[tcn][:, :, D : D + 1],
                    in_=maskv[:, tcn, None].to_broadcast((P, H, 1)),
                )

            # ---- per head-pair: QT/KT projection, S^T, exp, O^T, normalize.
            # The PE executes matmuls strictly in program order, so pair c+1's
            # Q/K projection matmuls are interleaved into pair c's S/exp phase
            # as fill work; pair 7 gets Wo-projection chains instead.
            def emit_qk(c):
                """Allocate pair c's QT/KT tiles; return (QTc, KTc, steps)."""
                QTc = pb.tile([P, TQ], BF16, tag="qtc", name=f"qt{c}")
                KTc = pb.tile([P, KL], BF16, tag="ktc", name=f"kt{c}")
                steps = []
                psq = psP.tile([P, TQ], F32, tag="psp", name=f"psq{c}")
                if QK_FP8:
                    for kc in range(0, NC, 2):
                        steps.append(
                            lambda kc=kc: nc.tensor.matmul(
                                psq[:],
                                Wq_sb[:, kc : kc + 2, c * P : (c + 1) * P],
                                xTq[:, kc : kc + 2, :],
                                start=(kc == 0),
                                stop=(kc == NC - 2),
                                perf_mode=DR,
                            )
                        )
                else:
                    for kc in range(NC):
                        steps.append(
                            lambda kc=kc: nc.tensor.matmul(
                                psq[:],
                                Wq_sb[:, kc, c * P : (c + 1) * P],
                                xTq[:, kc, :],
                                start=(kc == 0),
                                stop=(kc == NC - 1),
                            )
                        )
                steps.append(
                    lambda: nc.vector.tensor_scalar(
                        QTc[:], psq[:], vecs[:, c, 0:1], 1.0 / PSCALE,
                        OP.add, OP.mult,
                    )
                )
                for ko in range(0, KL, TQ):
                    w = min(TQ, KL - ko)
                    psk = psP.tile([P, TQ], F32, tag="psp", name=f"psk{c}{ko}")
                    if QK_FP8:
                        for kc in range(0, NC, 2):
                            steps.append(
                                lambda kc=kc, ko=ko, w=w, psk=psk: nc.tensor.matmul(
                                    psk[:, :w],
                                    Wk_sb[:, kc : kc + 2, c * P : (c + 1) * P],
                                    xT[:, kc : kc + 2, ko : ko + w],
                                    start=(kc == 0),
                                    stop=(kc == NC - 2),
                                    perf_mode=DR,
                                )
                            )
                    else:
                        for kc in range(NC):
                            steps.append(
                                lambda kc=kc, ko=ko, w=w, psk=psk: nc.tensor.matmul(
                                    psk[:, :w],
                                    Wk_sb[:, kc, c * P : (c + 1) * P],
                                    xT[:, kc, ko : ko + w],
                                    start=(kc == 0),
                                    stop=(kc == NC - 1),
                                )
                            )
                    steps.append(
                        lambda ko=ko, w=w, psk=psk: nc.vector.tensor_scalar(
                            KTc[:, ko : ko + w], psk[:, :w], vecs[:, c, 1:2],
                            1.0 / PSCALE, OP.add, OP.mult,
                        )
                    )
                return QTc, KTc, steps

            # ---- Wo chain machinery (chains scheduled as late-pair fill) ----
            def wo_make(qc, pse):
                """pse = [psum AP nn0, psum AP nn1]. Returns (mm, stat, finish)."""
                qs = slice(qc * P, (qc + 1) * P)
                stats = pb.tile(
                    [P, 2, nc.vector.BN_STATS_DIM], F32, tag="stats",
                    name=f"stats{qc}",
                )

                def mm(nn, mc):
                    # mc in {0, 2, 4}: DoubleRow over the OT pair tile;
                    # mc in {6, 7}: plain fp8 (keeps the last two OTs
                    # independently schedulable).
                    def f():
                        if mc == 0 and use_boe:
                            nc.tensor.matmul(
                                pse[nn],
                                ones_sb[:, 0:P],
                                boe_sb[:, nn * TQ : (nn + 1) * TQ],
                                start=True, stop=False,
                            )
                        if mc < 6 and WO_FP8:
                            nc.tensor.matmul(
                                pse[nn],
                                OTp[mc // 2][:, :, qs],
                                Wo_sb[:, mc : mc + 2, nn * TQ : (nn + 1) * TQ],
                                start=(mc == 0 and not use_boe),
                                stop=False,
                                perf_mode=DR,
                            )
                        elif mc < 6:
                            for m2 in (mc, mc + 1):
                                nc.tensor.matmul(
                                    pse[nn],
                                    OTp[m2 // 2][:, m2 % 2, qs],
                                    Wo_sb[:, m2, nn * TQ : (nn + 1) * TQ],
                                    start=(m2 == 0 and not use_boe),
                                    stop=False,
                                )
                        else:
                            nc.tensor.matmul(
                                pse[nn],
                                OTp[mc // 2][:, mc % 2, qs],
                                Wo_sb[:, mc, nn * TQ : (nn + 1) * TQ],
                                start=False,
                                stop=(mc == NC - 1),
                            )
                    return f

                def stat(nn):
                    def f():
                        nc.vector.bn_stats(out=stats[:, nn, :], in_=pse[nn])
                    return f

                def finish():
                    qsl = slice(qc * P, (qc + 1) * P)
                    mv = pb.tile([P, nc.vector.BN_AGGR_DIM], F32, tag="mv")
                    nc.vector.bn_aggr(out=mv[:], in_=stats[:])
                    rstd = pb.tile([P, 1], F32, tag="rstd")
                    nmr = pb.tile([P, 1], F32, tag="nmr")
                    nc.scalar.activation(
                        out=rstd[:], in_=mv[:, 1:2],
                        func=AF.Sqrt, bias=eps_t[:], scale=1.0,
                    )
                    nc.vector.reciprocal(out=rstd[:], in_=rstd[:])
                    nc.vector.tensor_scalar(
                        nmr[:], mv[:, 0:1], rstd[:], -1.0,
                        OP.mult, OP.mult,
                    )
                    Y = pb.tile([P, C], BF16, tag="ysb", bufs=2)
                    for nn in range(2):
                        cs = slice(nn * TQ, (nn + 1) * TQ)
                        if nn == 0:
                            nc.scalar.activation(
                                out=Y[:, cs], in_=pse[nn],
                                func=AF.Identity,
                                bias=nmr[:], scale=rstd[:],
                            )
                        else:
                            nc.vector.scalar_tensor_tensor(
                                Y[:, cs], pse[nn], rstd[:],
                                nmr[:, 0:1].to_broadcast((P, TQ)),
                                OP.mult, OP.add,
                            )
                        if use_lnw:
                            nc.vector.tensor_tensor(
                                Y[:, cs], Y[:, cs], lng_rep[:, cs], OP.mult
                            )
                            nc.gpsimd.tensor_tensor(
                                Y[:, cs], Y[:, cs], lnb_rep[:, cs], OP.add
                            )
                        eng = nc.sync if (2 * qc + nn) % 2 == 0 else nc.scalar
                        eng.dma_start(out=dr["y"].ap()[qsl, cs], in_=Y[:, cs])

                return mm, stat, finish

            wo = {}  # qc -> (mm, stat, finish)

            def make_osteps(po0, po1, ets, g0, g1):
                steps = []
                for jc in range(NK):
                    steps.append(
                        lambda jc=jc: nc.tensor.matmul(
                            po0[0 : D + 1, :], VA[jc][:, g0, :],
                            ets[jc][:, 0, :],
                            start=(jc == 0), stop=(jc == NK - 1),
                        )
                    )
                for jc in range(NK):
                    steps.append(
                        lambda jc=jc: nc.tensor.matmul(
                            po1[0 : D + 1, :], VA[jc][:, g1, :],
                            ets[jc][:, 1, :],
                            start=(jc == 0), stop=(jc == NK - 1),
                        )
                    )
                return steps

            def norm(c, po0, po1):
                # d rows live on psum partition 64: copy out, DMA-shift to
                # partition 0 (approx-recip ucode is broken at base!=0),
                # reciprocal, then DRAM-bounce partition broadcast.
                dsb = pb.tile([P, 2 * TQ], F32, tag="dsb")
                dp0 = pb.tile([1, 2 * TQ], F32, tag="dp0")
                rp0 = pb.tile([1, 2 * TQ], F32, tag="rp0")
                rrep = pb.tile([D, 2 * TQ], F32, tag="rrep")
                nc.vector.tensor_scalar_mul(
                    dsb[D : D + 1, 0:TQ], po0[D : D + 1, :], 1.0 / OSCALE
                )
                nc.vector.tensor_scalar_mul(
                    dsb[D : D + 1, TQ:], po1[D : D + 1, :], 1.0 / OSCALE
                )
                nc.sync.dma_start(out=dp0[0:1, :], in_=dsb[D : D + 1, :])
                nc.vector.reciprocal_approx_fast(out=rp0[:], in_=dp0[:])
                rdram = pbd.tile([1, 2 * TQ], F32, tag="rdram")
                nc.sync.dma_start(out=rdram[:], in_=rp0[0:1, :])
                src = rdram[0:1, :]
                bcast = bass.AP(
                    tensor=src.tensor, offset=src.offset, ap=[[0, D]] + src.ap[1:]
                )
                nc.sync.dma_start(out=rrep[:], in_=bcast)
                # normalize: even head straight into OT, odd staged + DMA shift
                odd = pb.tile([D, TQ], OT_T, tag="odd")
                nc.vector.tensor_tensor(
                    OTp[c // 2][0:D, c % 2, :], po0[0:D, :], rrep[:, 0:TQ], OP.mult
                )
                nc.vector.tensor_tensor(
                    odd[:], po1[0:D, :], rrep[:, TQ:], OP.mult
                )
                nc.sync.dma_start(out=OTp[c // 2][D:P, c % 2, :], in_=odd[:])

            QTc, KTc, steps = emit_qk(0)
            for st in steps:
                st()
            prev = None  # (c-1, po0, po1) awaiting normalize
            for c in range(NPAIR):
                h0, h1 = 2 * c, 2 * c + 1
                if c + 1 < NPAIR:
                    nQT, nKT, nsteps = emit_qk(c + 1)
                else:
                    nQT, nKT = None, None
                    # pair 7's extra fill = Wo chain qc0 (freed psP bufs)
                    pw0 = psP.tile([P, TQ], F32, tag="psp", name="wo0a")
                    pw1 = psP.tile([P, TQ], F32, tag="psp", name="wo0b")
                    wo[0] = wo_make(0, [pw0[:], pw1[:]])
                    nsteps = [wo[0][0](nn, mc) for nn in range(2) for mc in (0, 2, 4)]

                # this pair's O psums; its matmuls run as the NEXT pair's
                # fill (they depend only on already-finished exps, so they
                # are always-ready work during the serial exp chain).
                po0 = psO.tile([P, TQ], F32, tag="po0", bufs=2)
                po1 = psO.tile([P, TQ], F32, tag="po1", bufs=2)
                ets = [
                    pb.tile([P, 2, TQ], BF16, tag=f"et{jc}", bufs=2, name=f"et{jc}")
                    for jc in range(NK)
                ]
                osteps = make_osteps(po0, po1, ets, h0, h1)
                if c + 1 < NPAIR:
                    # QK fill first (its results gate the next pair's S);
                    # the previous pair's O steps absorb the late slots.
                    if prev is not None:
                        nsteps = nsteps + prev[3]
                else:
                    if prev is not None:
                        nsteps = prev[3] + nsteps
                    nsteps = nsteps + osteps

                nfill = len(nsteps)
                cuts = [0]
                for jc in range(NK):
                    frac = 0 if jc == 0 else jc / (NK - 1)
                    cuts.append(round(nfill * frac))
                for jc in range(NK):
                    js = slice(jc * P, (jc + 1) * P)
                    s01 = psS.tile([P, 2, TQ], F32, tag="s01", bufs=1)
                    nc.tensor.matmul(
                        s01[:, 0, :],
                        KTc[0:D, js],
                        QTc[0:D, :],
                        start=True, stop=True,
                        tile_position=(0, 0),
                    )
                    nc.tensor.matmul(
                        s01[:, 1, :],
                        KTc[D:P, js],
                        QTc[D:P, :],
                        start=True, stop=True,
                        tile_position=(D, 0),
                    )
                    nc.scalar.activation(
                        out=ets[jc][:], in_=s01[:],
                        func=AF.Exp, scale=0.125,
                    )
                    for st in nsteps[cuts[jc] : cuts[jc + 1]]:
                        st()
                for st in nsteps[cuts[NK] :]:
                    st()

                if prev is not None:
                    norm(prev[0], prev[1], prev[2])
                prev = (c, po0, po1, osteps)
                QTc, KTc = nQT, nKT
            norm(prev[0], prev[1], prev[2])

            # ---- remaining output-projection chains + LN. qc2 takes the
            # s01 bufs (free after pair 7's exps); qc3 reuses qc0's psP
            # bufs once qc0's LN apply has drained them. Finishers are
            # interleaved so the applies overlap later chains' matmuls.
            pw1a = psO.tile([P, TQ], F32, tag="po0", bufs=2, name="wo1a")
            pw1b = psO.tile([P, TQ], F32, tag="po1", bufs=2, name="wo1b")
            wo[1] = wo_make(1, [pw1a[:], pw1b[:]])
            for nn in range(2):
                for mc in (0, 2, 4):
                    wo[1][0](nn, mc)()
            for q in (0, 1):
                for nn in range(2):
                    wo[q][0](nn, 6)()
            sw2 = psS.tile([P, 2, TQ], F32, tag="s01", bufs=1, name="wo2")
            wo[2] = wo_make(2, [sw2[:, 0, :], sw2[:, 1, :]])
            for nn in range(2):
                for mc in (0, 2, 4, 6):
                    wo[2][0](nn, mc)()
            pw3a = psO.tile([P, TQ], F32, tag="po0", bufs=2, name="wo3a")
            pw3b = psO.tile([P, TQ], F32, tag="po1", bufs=2, name="wo3b")
            wo[3] = wo_make(3, [pw3a[:], pw3b[:]])
            for nn in range(2):
                for mc in (0, 2, 4, 6):
                    wo[3][0](nn, mc)()
            # finishers: last accumulation (needs pair 7's OT), stats, LN
            for q in (0, 1, 2, 3):
                mm, stat, finish = wo[q]
                for nn in range(2):
                    mm(nn, 7)()
                    stat(nn)()
                finish()


def _build(NK=NT, use_lnw=True, use_boe=True):
    nc = bacc.Bacc("TRN2", target_bir_lowering=False, debug=False, num_devices=8)
    dr = {}
    dr["xT"] = nc.dram_tensor("xT", [P, NC, NK * P], PROJ_T, kind="ExternalInput")
    dr["xTq"] = nc.dram_tensor("xTq", [P, NC, TQ], PROJ_T, kind="ExternalInput")
    for w in ("Wq", "Wk"):
        dr[w] = nc.dram_tensor(w, [P, NC, C], PROJ_T, kind="ExternalInput")
    dr["Wo"] = nc.dram_tensor("Wo", [P, NC, C], OT_T, kind="ExternalInput")
    dr["Wv"] = nc.dram_tensor("Wv", [P, 2, NC, TQ], PROJ_T, kind="ExternalInput")
    dr["vecs"] = nc.dram_tensor("vecs", [P, NC, 3], F32, kind="ExternalInput")
    dr["maskv"] = nc.dram_tensor("maskv", [P, NK], BF16, kind="ExternalInput")
    if use_boe:
        dr["boe"] = nc.dram_tensor("boe", [1, C], BF16, kind="ExternalInput")
    if use_lnw:
        for v in ("lng", "lnb"):
            dr[v] = nc.dram_tensor(v, [1, C], F32, kind="ExternalInput")
    dr["y"] = nc.dram_tensor("y", [TQ, C], BF16, kind="ExternalOutput")
    with tile.TileContext(nc) as tc:
        _emit(nc, tc, dr, NK, use_lnw, use_boe)
    nc.compile()
    return nc


def _chunk(a):
    """[C, N] -> [128, C//128, N] with [p, c, n] = a[128c+p, n]."""
    return np.ascontiguousarray(
        a.reshape(NC, P, -1).transpose(1, 0, 2)
    )


def _prep_inputs(inputs):
    f32 = np.float32
    Wq = np.asarray(inputs["Wq"], f32)
    Wk = np.asarray(inputs["Wk"], f32)
    Wv = np.asarray(inputs["Wv"], f32)
    Wo = np.asarray(inputs["Wo"], f32)
    x = np.asarray(inputs["x"], f32)
    mask = np.asarray(inputs["attn_mask"]).reshape(B, T)
    # sort keys so unmasked come first; masked tail chunks are dropped
    perms = [np.argsort(-mask[b], kind="stable") for b in range(B)]
    m1max = max(int(mask[b].sum()) for b in range(B))
    NK = min(NT, max(1, -(-m1max // P)))
    KL = NK * P
    bq = np.asarray(inputs["bq"], f32)
    bk = np.asarray(inputs["bk"], f32)
    bv = np.asarray(inputs["bv"], f32)
    bo = np.asarray(inputs["bo"], f32)
    ln_g = np.asarray(inputs["ln_g"], f32)
    ln_b = np.asarray(inputs["ln_b"], f32)

    boe = (bv @ Wo + bo).astype(f32)
    use_boe = bool(np.any(boe != 0.0))
    use_lnw = bool(np.any(ln_g != 1.0) or np.any(ln_b != 0.0))

    def _fp8(a):
        return np.clip(a, -240.0, 240.0).astype(NPFP8)

    def _proj(a):
        return _fp8(a * WSCALE) if QK_FP8 else a.astype(NPBF16)

    def _projx(a):
        return _fp8(a) if QK_FP8 else a.astype(NPBF16)

    shared = {
        "Wq": _proj(_chunk(Wq)),
        "Wk": _proj(_chunk(Wk)),
        "Wv": _proj(
            np.ascontiguousarray(
                _chunk(Wv).reshape(P, NC, 2, TQ).transpose(0, 2, 1, 3)
            )
        ),
        "Wo": _fp8(_chunk(Wo) * WSCALE)
        if WO_FP8
        else _chunk(Wo).astype(NPBF16),
    }
    if use_boe:
        bsc = (WSCALE * OSCALE) if WO_FP8 else 1.0
        shared["boe"] = (boe * bsc).reshape(1, C).astype(NPBF16)
    if use_lnw:
        shared["lng"] = ln_g.reshape(1, C).astype(f32)
        shared["lnb"] = ln_b.reshape(1, C).astype(f32)
    in_maps = []
    for core in range(8):
        b, half = core // 2, core % 2
        xt = np.ascontiguousarray(x[b].T)  # [C, T]
        pk = perms[b][:KL]
        mfp = mask[b][pk].astype(f32)     # permuted/truncated key mask
        vcol = np.zeros((P, NC), f32)
        vcol[:, :NK] = mfp.reshape(NK, P).T / PSCALE
        vecs = np.stack(
            [
                bq.reshape(NC, P).T * PSCALE,
                bk.reshape(NC, P).T * PSCALE,
                vcol,
            ],
            axis=-1,
        )
        m = dict(shared)
        m["xT"] = _projx(_chunk(np.ascontiguousarray(xt[:, pk])))
        m["xTq"] = _projx(_chunk(xt[:, half * TQ : (half + 1) * TQ]))
        m["vecs"] = np.ascontiguousarray(vecs, f32)
        m["maskv"] = np.ascontiguousarray(mfp.reshape(NK, P).T.astype(NPBF16))
        in_maps.append(m)
    return NK, use_lnw, use_boe, in_maps


def kernel(**inputs):
    global LAST_RESULTS
    NK, use_lnw, use_boe, in_maps = _prep_inputs(inputs)
    key = (
        "nc", NK, use_lnw, use_boe,
        os.environ.get("BASS_NWARM", "12"), WO_FP8, QK_FP8,
    )
    if key not in _CACHE:
        _CACHE[key] = _build(NK=NK, use_lnw=use_lnw, use_boe=use_boe)
    nc = _CACHE[key]

    trace = os.environ.get("KERNEL_TRACE", "0") == "1"
    if trace:
        _ensure_ntff_hook()
    LAST_RESULTS = run_bass_kernel_spmd(
        nc, in_maps, core_ids=list(range(8)), trace=trace
    )
    out = np.empty((B, T, C), np.float32)
    for core in range(8):
        b, half = core // 2, core % 2
        out[b, half * TQ : (half + 1) * TQ, :] = np.asarray(
            LAST_RESULTS.results[core]["y"], dtype=np.float32
        )
    return out


# revision 62
# speedup vs baseline: 1.0607x; 1.0607x over previous
"""Trainium2 Bass kernel for nn_MultiHeadAttention (B=4,T=1024,C=1024,H=16).

Sharding: 8 cores = 4 batches x 2 query-halves. Each core computes, for its
batch b and its 512 query rows:
  V projection (natural layout, mask folded in, +mask column for denominator),
  then per head-pair: Q^T/K^T projection chunks, S^T = K^T.T @ Q^T (row-tiled
  head pairs, D=64 contraction), one exp ACT per key chunk over the merged
  two-head S psum, O^T+denominator via one augmented matmul lhsT=[V_h*m | m],
  normalize via reciprocal + DRAM-bounce partition-broadcast; finally
  Y = O^T.T @ Wo with LN stats read straight from PSUM and the LN apply on
  ACT. Keys are mask-sorted host-side so masked tail chunks drop (NK=5 of 8).

Perf notes (HW ~152us baseline -> ~140-142us this version, cool chip;
sustained back-to-back runs can thermally throttle the PE to ~165us):
  - PE HAM clock gate: the PE runs at 1.2GHz until ~3.4us of sustained
    activity and re-throttles after idle windows, so idle gaps cost double.
    NWARM warmup matmuls on a zeroed tile hold the clock gate open while
    the first input DMAs stream (first real matmul ~8us vs ~16us before).
  - each pair's O^T matmuls are emitted as the NEXT pair's fill work
    (after its QK fill): they depend only on already-finished exps, so
    they are always-ready cover for the serial per-chunk exp chain.
  - Wo-projection chain qc0 (on the freed psP bufs) is emitted as pair 7's
    fill work; qc1 (po bufs), qc2 (s01 bufs), qc3 (po bufs again) follow
    right after the pair loop, each on disjoint psum banks so no chain
    waits on another's LN apply; all four finishers run at the end where
    pair 7's normalize has certainly drained.
  - only sync+scalar have fast (~105 GB/s each) HW DMA queues; V-projection
    inputs land kc-interleaved across both queues in first-use order so
    the first chains trickle along with the DMA instead of stalling on a
    full half; the remaining tensors stream as chunk-halves.
  - fp8/DoubleRow paths exist behind BASS_QK_FP8/BASS_WO_FP8 but are OFF:
    e4m3 quantization injects ~2-4% relative noise into every dot product
    (random-sign sums keep per-element relative error), busting the 2e-2
    gate (measured 4.3e-2 with fp8 projections).
Host gathers the 8 [512,1024] outputs into [4,1024,1024].
"""

import os
import sys

import numpy as np

for _p in ("/opt/trn_rl_repo", "/root/.axon_site/_ro/trn_rl_repo"):
    if os.path.isdir(_p) and _p not in sys.path:
        sys.path.append(_p)

import ml_dtypes  # noqa: E402
import concourse.bass as bass  # noqa: E402
import concourse.mybir as mybir  # noqa: E402
import concourse.tile as tile  # noqa: E402
from concourse import bacc  # noqa: E402
from concourse.bass_utils import run_bass_kernel_spmd  # noqa: E402

BF16 = mybir.dt.bfloat16
FP8 = mybir.dt.float8e4
F32 = mybir.dt.float32
NPBF16 = ml_dtypes.bfloat16
NPFP8 = ml_dtypes.float8_e4m3fn
DR = mybir.MatmulPerfMode.DoubleRow
WSCALE = 64.0     # weights stored x64 in fp8 (keeps them out of subnormals)
WO_FP8 = os.environ.get("BASS_WO_FP8", "0") == "1"
QK_FP8 = os.environ.get("BASS_QK_FP8", "0") == "1"
PROJ_T = FP8 if QK_FP8 else BF16
PSCALE = WSCALE if QK_FP8 else 1.0
OSCALE = 16.0 if WO_FP8 else 1.0  # normalized O^T stored x16 in fp8 (LN absorbs it)
OT_T = FP8 if WO_FP8 else BF16

B, T, C, H = 4, 1024, 1024, 16
D = C // H          # 64
P = 128             # partitions
NC = C // P         # 8 chunks of C
NT = T // P         # 8 chunks of T
TQ = T // 2         # 512 query rows per core
NQ = TQ // P        # 4 query chunks
NPAIR = H // 2      # 8 head pairs
EPS = 1e-5

_CACHE = {}
LAST_RESULTS = None


def _ensure_ntff_hook():
    """Register the axon NTFF profiling hook if the image's antenv lacks it."""
    try:
        import antenv.axon_hooks  # noqa: F401
        return
    except ImportError:
        pass
    try:
        import types

        import antenv
        from trn_agent_boot.trn_boot import _ntff_profile_via_ctypes

        mod = types.ModuleType("antenv.axon_hooks")
        state = {"hook": None}
        mod.set_axon_ntff_profile_hook = lambda h: state.__setitem__("hook", h)
        mod.get_axon_ntff_profile_hook = lambda: state["hook"]
        sys.modules["antenv.axon_hooks"] = mod
        antenv.axon_hooks = mod
        hook = _ntff_profile_via_ctypes("/opt/axon/libaxon_pjrt.so")
        if hook is not None:
            mod.set_axon_ntff_profile_hook(hook)
    except Exception:
        pass


def _emit(nc, tc, dr, NK, use_lnw, use_boe):
    """Emit the per-core Tile program (projections interleaved with attention)."""
    from contextlib import ExitStack

    AF = mybir.ActivationFunctionType
    OP = mybir.AluOpType
    NWARM = int(os.environ.get("BASS_NWARM", "12"))

    with ExitStack() as ctx:
        consts = ctx.enter_context(tc.tile_pool(name="consts", bufs=1))

        # ---- persistent SBUF tiles ----
        KL = NK * P
        VA = [
            consts.tile([P, H, D + 1], BF16, tag=f"va{j}", name=f"va{j}")
            for j in range(NK)
        ]
        # OT pair tiles: OTp[j][:, m, :] = head-pair 2j+m's normalized O^T
        OTp = [
            consts.tile([P, 2, TQ], OT_T, tag=f"otp{j}", name=f"otp{j}")
            for j in range(NPAIR // 2)
        ]
        Wo_sb = consts.tile([P, NC, C], OT_T)
        vecs = consts.tile([P, NC, 3], F32)        # bq | bk | maskf
        maskv = consts.tile([P, NK], BF16)
        eps_t = consts.tile([P, 1], F32)
        warm = consts.tile([P, TQ], BF16)
        if use_lnw:
            lng_rep = consts.tile([P, C], F32)
            lnb_rep = consts.tile([P, C], F32)
        if use_boe:
            boe_sb = consts.tile([1, C], BF16)     # bv@Wo+bo row (partition 0)
            ones_sb = consts.tile([1, P], BF16)    # ones row for bias preload
            nc.vector.memset(ones_sb, 1.0)

        nc.vector.memset(eps_t, EPS)
        nc.vector.memset(warm, 0.0)

        with (
            tc.tile_pool(name="pa", bufs=1) as pa,
            tc.tile_pool(name="pb", bufs=2) as pb,
            tc.tile_pool(name="pbd", bufs=2, space="DRAM") as pbd,
            tc.tile_pool(name="psP", bufs=2, space="PSUM") as psP,
            tc.tile_pool(name="psS", bufs=2, space="PSUM") as psS,
            tc.tile_pool(name="psO", bufs=2, space="PSUM") as psO,
        ):
            # ---- PE warmup: hold the HAM clock gate open while the input
            # DMAs stream; results are never read.
            wps = psS.tile([P, 2, TQ], F32, tag="s01", bufs=1, name="warmps")
            for _ in range(NWARM):
                nc.tensor.matmul(
                    wps[:, 0, :], warm[:, 0:P], warm[:], start=True, stop=True
                )

            xT = pa.tile([P, NC, KL], PROJ_T)
            xTq = pa.tile([P, NC, TQ], PROJ_T)
            Wq_sb = pa.tile([P, NC, C], PROJ_T)
            Wk_sb = pa.tile([P, NC, C], PROJ_T)
            Wv_sb = pa.tile([P, 2, NC, TQ], PROJ_T)  # nn-major

            # ---- input DMAs. Only sync and scalar have fast HW queues;
            # gpsimd's software queue only carries the small vectors. Each
            # tensor is split into two chunk-halves, one per queue, ordered
            # by first use: V-projection inputs first, Wo last.
            HNC = NC // 2
            xT_a = dr["xT"].ap()
            wv0_a = dr["Wv"].ap()[:, 0]
            wv1_a = dr["Wv"].ap()[:, 1]
            # V-projection inputs land kc-interleaved so the first chains
            # trickle along with the DMA instead of stalling on a full half.
            for eng, lo in ((nc.sync, 0), (nc.scalar, 4)):
                eng.dma_start(out=xT[:, lo : lo + 2], in_=xT_a[:, lo : lo + 2])
                eng.dma_start(
                    out=Wv_sb[:, 0, lo : lo + 2], in_=wv0_a[:, lo : lo + 2]
                )
                eng.dma_start(out=xT[:, lo + 2 : lo + 4], in_=xT_a[:, lo + 2 : lo + 4])
                eng.dma_start(
                    out=Wv_sb[:, 0, lo + 2 : lo + 4], in_=wv0_a[:, lo + 2 : lo + 4]
                )
                eng.dma_start(
                    out=Wv_sb[:, 1, lo : lo + 4], in_=wv1_a[:, lo : lo + 4]
                )
            halves = [
                ("xTq", xTq, xTq),
                ("Wq", Wq_sb, Wq_sb), ("Wk", Wk_sb, Wk_sb),
                ("Wo", Wo_sb, Wo_sb),
            ]
            for name, tl, _ in halves:
                a = dr[name].ap()
                nc.sync.dma_start(out=tl[:, 0:HNC], in_=a[:, 0:HNC])
                nc.scalar.dma_start(out=tl[:, HNC:], in_=a[:, HNC:])
            nc.gpsimd.dma_start(out=vecs[:], in_=dr["vecs"].ap()[:])
            nc.gpsimd.dma_start(out=maskv[:], in_=dr["maskv"].ap()[:])
            if use_boe:
                nc.gpsimd.dma_start(out=boe_sb[:], in_=dr["boe"].ap()[:])
            if use_lnw:
                for name, rep in (("lng", lng_rep), ("lnb", lnb_rep)):
                    a = dr[name].ap()
                    bcast = bass.AP(
                        tensor=a.tensor, offset=a.offset, ap=[[0, P], [1, C]]
                    )
                    nc.gpsimd.dma_start(out=rep[:], in_=bcast)

            # ---- V projection: natural [KL, C], masked rows, + mask col ----
            # nn-outer so the first 5 groups only need Wv's nn=0 columns.
            for nn in range(2):
                for tcn in range(NK):
                    ps = psP.tile([P, TQ], F32, tag="psp")
                    if QK_FP8:
                        for kc in range(0, NC, 2):
                            nc.tensor.matmul(
                                ps[:],
                                xT[:, kc : kc + 2, tcn * P : (tcn + 1) * P],
                                Wv_sb[:, nn, kc : kc + 2, :],
                                start=(kc == 0),
                                stop=(kc == NC - 2),
                                perf_mode=DR,
                            )
                    else:
                        for kc in range(NC):
                            nc.tensor.matmul(
                                ps[:],
                                xT[:, kc, tcn * P : (tcn + 1) * P],
                                Wv_sb[:, nn, kc, :],
                                start=(kc == 0),
                                stop=(kc == NC - 1),
                            )
                    nc.vector.tensor_scalar_mul(
                        VA[tcn][:, nn * 8 : (nn + 1) * 8, 0:D],
                        ps[:].rearrange("p (h d) -> p h d", h=8),
                        vecs[:, tcn, 2:3],
                    )
            for tcn in range(NK):
                nc.vector.tensor_copy(
                    out=VA[tcn][:, :, D : D + 1],
                    in_=maskv[:, tcn, None].to_broadcast((P, H, 1)),
                )

            # ---- per head-pair: QT/KT projection, S^T, exp, O^T, normalize.
            # The PE executes matmuls strictly in program order, so pair c+1's
            # Q/K projection matmuls are interleaved into pair c's S/exp phase
            # as fill work; pair 7 gets Wo-projection chains instead.
            def emit_qk(c):
                """Allocate pair c's QT/KT tiles; return (QTc, KTc, steps)."""
                QTc = pb.tile([P, TQ], BF16, tag="qtc", name=f"qt{c}")
                KTc = pb.tile([P, KL], BF16, tag="ktc", name=f"kt{c}")
                steps = []
                psq = psP.tile([P, TQ], F32, tag="psp", name=f"psq{c}")
                if QK_FP8:
                    for kc in range(0, NC, 2):
                        steps.append(
                            lambda kc=kc: nc.tensor.matmul(
                                psq[:],
                                Wq_sb[:, kc : kc + 2, c * P : (c + 1) * P],
                                xTq[:, kc : kc + 2, :],
                                start=(kc == 0),
                                stop=(kc == NC - 2),
                                perf_mode=DR,
                            )
                        )
                else:
                    for kc in range(NC):
                        steps.append(
                            lambda kc=kc: nc.tensor.matmul(
                                psq[:],
                                Wq_sb[:, kc, c * P : (c + 1) * P],
                                xTq[:, kc, :],
                                start=(kc == 0),
                                stop=(kc == NC - 1),
                            )
                        )
                steps.append(
                    lambda: nc.vector.tensor_scalar(
                        QTc[:], psq[:], vecs[:, c, 0:1], 1.0 / PSCALE,
                        OP.add, OP.mult,
                    )
                )
                for ko in range(0, KL, TQ):
                    w = min(TQ, KL - ko)
                    psk = psP.tile([P, TQ], F32, tag="psp", name=f"psk{c}{ko}")
                    if QK_FP8:
                        for kc in range(0, NC, 2):
                            steps.append(
                                lambda kc=kc, ko=ko, w=w, psk=psk: nc.tensor.matmul(
                                    psk[:, :w],
                                    Wk_sb[:, kc : kc + 2, c * P : (c + 1) * P],
                                    xT[:, kc : kc + 2, ko : ko + w],
                                    start=(kc == 0),
                                    stop=(kc == NC - 2),
                                    perf_mode=DR,
                                )
                            )
                    else:
                        for kc in range(NC):
                            steps.append(
                                lambda kc=kc, ko=ko, w=w, psk=psk: nc.tensor.matmul(
                                    psk[:, :w],
                                    Wk_sb[:, kc, c * P : (c + 1) * P],
                                    xT[:, kc, ko : ko + w],
                                    start=(kc == 0),
                                    stop=(kc == NC - 1),
                                )
                            )
                    steps.append(
                        lambda ko=ko, w=w, psk=psk: nc.vector.tensor_scalar(
                            KTc[:, ko : ko + w], psk[:, :w], vecs[:, c, 1:2],
                            1.0 / PSCALE, OP.add, OP.mult,
                        )
                    )
                return QTc, KTc, steps

            # ---- Wo chain machinery (chains scheduled as late-pair fill) ----
            def wo_make(qc, pse):
                """pse = [psum AP nn0, psum AP nn1]. Returns (mm, stat, finish)."""
                qs = slice(qc * P, (qc + 1) * P)
                stats = pb.tile(
                    [P, 2, nc.vector.BN_STATS_DIM], F32, tag="stats",
                    name=f"stats{qc}",
                )

                def mm(nn, mc):
                    # mc in {0, 2, 4}: DoubleRow over the OT pair tile;
                    # mc in {6, 7}: plain fp8 (keeps the last two OTs
                    # independently schedulable).
                    def f():
                        if mc == 0 and use_boe:
                            nc.tensor.matmul(
                                pse[nn],
                                ones_sb[:, 0:P],
                                boe_sb[:, nn * TQ : (nn + 1) * TQ],
                                start=True, stop=False,
                            )
                        if mc < 6 and WO_FP8:
                            nc.tensor.matmul(
                                pse[nn],
                                OTp[mc // 2][:, :, qs],
                                Wo_sb[:, mc : mc + 2, nn * TQ : (nn + 1) * TQ],
                                start=(mc == 0 and not use_boe),
                                stop=False,
                                perf_mode=DR,
                            )
                        elif mc < 6:
                            for m2 in (mc, mc + 1):
                                nc.tensor.matmul(
                                    pse[nn],
                                    OTp[m2 // 2][:, m2 % 2, qs],
                                    Wo_sb[:, m2, nn * TQ : (nn + 1) * TQ],
                                    start=(m2 == 0 and not use_boe),
                                    stop=False,
                                )
                        else:
                            nc.tensor.matmul(
                                pse[nn],
                                OTp[mc // 2][:, mc % 2, qs],
                                Wo_sb[:, mc, nn * TQ : (nn + 1) * TQ],
                                start=False,
                                stop=(mc == NC - 1),
                            )
                    return f

                def stat(nn):
                    def f():
                        nc.vector.bn_stats(out=stats[:, nn, :], in_=pse[nn])
                    return f

                def finish():
                    qsl = slice(qc * P, (qc + 1) * P)
                    mv = pb.tile([P, nc.vector.BN_AGGR_DIM], F32, tag="mv")
                    nc.vector.bn_aggr(out=mv[:], in_=stats[:])
                    rstd = pb.tile([P, 1], F32, tag="rstd")
                    nmr = pb.tile([P, 1], F32, tag="nmr")
                    nc.scalar.activation(
                        out=rstd[:], in_=mv[:, 1:2],
                        func=AF.Sqrt, bias=eps_t[:], scale=1.0,
                    )
                    nc.vector.reciprocal(out=rstd[:], in_=rstd[:])
                    nc.vector.tensor_scalar(
                        nmr[:], mv[:, 0:1], rstd[:], -1.0,
                        OP.mult, OP.mult,
                    )
                    Y = pb.tile([P, C], BF16, tag="ysb", bufs=2)
                    for nn in range(2):
                        cs = slice(nn * TQ, (nn + 1) * TQ)
                        if nn == 0:
                            nc.scalar.activation(
                                out=Y[:, cs], in_=pse[nn],
                                func=AF.Identity,
                                bias=nmr[:], scale=rstd[:],
                            )
                        else:
                            nc.vector.scalar_tensor_tensor(
                                Y[:, cs], pse[nn], rstd[:],
                                nmr[:, 0:1].to_broadcast((P, TQ)),
                                OP.mult, OP.add,
                            )
                        if use_lnw:
                            nc.vector.tensor_tensor(
                                Y[:, cs], Y[:, cs], lng_rep[:, cs], OP.mult
                            )
                            nc.gpsimd.tensor_tensor(
                                Y[:, cs], Y[:, cs], lnb_rep[:, cs], OP.add
                            )
                        eng = nc.sync if (2 * qc + nn) % 2 == 0 else nc.scalar
                        eng.dma_start(out=dr["y"].ap()[qsl, cs], in_=Y[:, cs])

                return mm, stat, finish

            wo = {}  # qc -> (mm, stat, finish)

            def make_osteps(po0, po1, ets, g0, g1):
                steps = []
                for jc in range(NK):
                    steps.append(
                        lambda jc=jc: nc.tensor.matmul(
                            po0[0 : D + 1, :], VA[jc][:, g0, :],
                            ets[jc][:, 0, :],
                            start=(jc == 0), stop=(jc == NK - 1),
                        )
                    )
                for jc in range(NK):
                    steps.append(
                        lambda jc=jc: nc.tensor.matmul(
                            po1[0 : D + 1, :], VA[jc][:, g1, :],
                            ets[jc][:, 1, :],
                            start=(jc == 0), stop=(jc == NK - 1),
                        )
                    )
                return steps

            def norm(c, po0, po1):
                # d rows live on psum partition 64: copy out, DMA-shift to
                # partition 0 (approx-recip ucode is broken at base!=0),
                # reciprocal, then DRAM-bounce partition broadcast.
                dsb = pb.tile([P, 2 * TQ], F32, tag="dsb")
                dp0 = pb.tile([1, 2 * TQ], F32, tag="dp0")
                rp0 = pb.tile([1, 2 * TQ], F32, tag="rp0")
                rrep = pb.tile([D, 2 * TQ], F32, tag="rrep")
                nc.vector.tensor_scalar_mul(
                    dsb[D : D + 1, 0:TQ], po0[D : D + 1, :], 1.0 / OSCALE
                )
                nc.vector.tensor_scalar_mul(
                    dsb[D : D + 1, TQ:], po1[D : D + 1, :], 1.0 / OSCALE
                )
                nc.sync.dma_start(out=dp0[0:1, :], in_=dsb[D : D + 1, :])
                nc.vector.reciprocal_approx_fast(out=rp0[:], in_=dp0[:])
                rdram = pbd.tile([1, 2 * TQ], F32, tag="rdram")
                nc.sync.dma_start(out=rdram[:], in_=rp0[0:1, :])
                src = rdram[0:1, :]
                bcast = bass.AP(
                    tensor=src.tensor, offset=src.offset, ap=[[0, D]] + src.ap[1:]
                )
                nc.sync.dma_start(out=rrep[:], in_=bcast)
                # normalize: even head straight into OT, odd staged + DMA shift
                odd = pb.tile([D, TQ], OT_T, tag="odd")
                nc.vector.tensor_tensor(
                    OTp[c // 2][0:D, c % 2, :], po0[0:D, :], rrep[:, 0:TQ], OP.mult
                )
                nc.vector.tensor_tensor(
                    odd[:], po1[0:D, :], rrep[:, TQ:], OP.mult
                )
                nc.sync.dma_start(out=OTp[c // 2][D:P, c % 2, :], in_=odd[:])

            QTc, KTc, steps = emit_qk(0)
            for st in steps:
                st()
            prev = None  # (c-1, po0, po1) awaiting normalize
            for c in range(NPAIR):
                h0, h1 = 2 * c, 2 * c + 1
                if c + 1 < NPAIR:
                    nQT, nKT, nsteps = emit_qk(c + 1)
                else:
                    nQT, nKT = None, None
                    # pair 7's extra fill = Wo chain qc0 (freed psP bufs)
                    pw0 = psP.tile([P, TQ], F32, tag="psp", name="wo0a")
                    pw1 = psP.tile([P, TQ], F32, tag="psp", name="wo0b")
                    wo[0] = wo_make(0, [pw0[:], pw1[:]])
                    nsteps = [wo[0][0](nn, mc) for nn in range(2) for mc in (0, 2, 4)]

                # this pair's O psums; its matmuls run as the NEXT pair's
                # fill (they depend only on already-finished exps, so they
                # are always-ready work during the serial exp chain).
                po0 = psO.tile([P, TQ], F32, tag="po0", bufs=2)
                po1 = psO.tile([P, TQ], F32, tag="po1", bufs=2)
                ets = [
                    pb.tile([P, 2, TQ], BF16, tag=f"et{jc}", bufs=2, name=f"et{jc}")
                    for jc in range(NK)
                ]
                osteps = make_osteps(po0, po1, ets, h0, h1)
                if c + 1 < NPAIR:
                    # QK fill first (its results gate the next pair's S);
                    # the previous pair's O steps absorb the late slots.
                    if prev is not None:
                        nsteps = nsteps + prev[3]
                else:
                    if prev is not None:
                        nsteps = prev[3] + nsteps
                    nsteps = nsteps + osteps

                nfill = len(nsteps)
                cuts = [0]
                for jc in range(NK):
                    frac = 0 if jc == 0 else jc / (NK - 1)
                    cuts.append(round(nfill * frac))
                for jc in range(NK):
                    js = slice(jc * P, (jc + 1) * P)
                    s01 = psS.tile([P, 2, TQ], F32, tag="s01", bufs=1)
                    nc.tensor.matmul(
                        s01[:, 0, :],
                        KTc[0:D, js],
                        QTc[0:D, :],
                        start=True, stop=True,
                        tile_position=(0, 0),
                    )
                    nc.tensor.matmul(
                        s01[:, 1, :],
                        KTc[D:P, js],
                        QTc[D:P, :],
                        start=True, stop=True,
                        tile_position=(D, 0),
                    )
                    nc.scalar.activation(
                        out=ets[jc][:], in_=s01[:],
                        func=AF.Exp, scale=0.125,
                    )
                    for st in nsteps[cuts[jc] : cuts[jc + 1]]:
                        st()
                for st in nsteps[cuts[NK] :]:
                    st()

                if prev is not None:
                    norm(prev[0], prev[1], prev[2])
                prev = (c, po0, po1, osteps)
                QTc, KTc = nQT, nKT
            norm(prev[0], prev[1], prev[2])

            # ---- remaining output-projection chains + LN. qc2 takes the
            # s01 bufs (free after pair 7's exps); qc3 reuses qc0's psP
            # bufs once qc0's LN apply has drained them. Finishers are
            # interleaved so the applies overlap later chains' matmuls.
            pw1a = psO.tile([P, TQ], F32, tag="po0", bufs=2, name="wo1a")
            pw1b = psO.tile([P, TQ], F32, tag="po1", bufs=2, name="wo1b")
            wo[1] = wo_make(1, [pw1a[:], pw1b[:]])
            for nn in range(2):
                for mc in (0, 2, 4):
                    wo[1][0](nn, mc)()
            for q in (0, 1):
                for nn in range(2):
                    wo[q][0](nn, 6)()
            sw2 = psS.tile([P, 2, TQ], F32, tag="s01", bufs=1, name="wo2")
            wo[2] = wo_make(2, [sw2[:, 0, :], sw2[:, 1, :]])
            for nn in range(2):
                for mc in (0, 2, 4, 6):
                    wo[2][0](nn, mc)()
            pw3a = psO.tile([P, TQ], F32, tag="po0", bufs=2, name="wo3a")
            pw3b = psO.tile([P, TQ], F32, tag="po1", bufs=2, name="wo3b")
            wo[3] = wo_make(3, [pw3a[:], pw3b[:]])
            for nn in range(2):
                for mc in (0, 2, 4, 6):
                    wo[3][0](nn, mc)()
            # finishers: last accumulation (needs pair 7's OT), stats, LN
            for q in (0, 1, 2, 3):
                mm, stat, finish = wo[q]
                for nn in range(2):
                    mm(nn, 7)()
                    stat(nn)()
                finish()


def _build(NK=NT, use_lnw=True, use_boe=True):
    nc = bacc.Bacc("TRN2", target_bir_lowering=False, debug=False, num_devices=8)
    dr = {}
    dr["xT"] = nc.dram_tensor("xT", [P, NC, NK * P], PROJ_T, kind="ExternalInput")
    dr["xTq"] = nc.dram_tensor("xTq", [P, NC, TQ], PROJ_T, kind="ExternalInput")
    for w in ("Wq", "Wk"):
        dr[w] = nc.dram_tensor(w, [P, NC, C], PROJ_T, kind="ExternalInput")
    dr["Wo"] = nc.dram_tensor("Wo", [P, NC, C], OT_T, kind="ExternalInput")
    dr["Wv"] = nc.dram_tensor("Wv", [P, 2, NC, TQ], PROJ_T, kind="ExternalInput")
    dr["vecs"] = nc.dram_tensor("vecs", [P, NC, 3], F32, kind="ExternalInput")
    dr["maskv"] = nc.dram_tensor("maskv", [P, NK], BF16, kind="ExternalInput")
    if use_boe:
        dr["boe"] = nc.dram_tensor("boe", [1, C], BF16, kind="ExternalInput")
    if use_lnw:
        for v in ("lng", "lnb"):
            dr[v] = nc.dram_tensor(v, [1, C], F32, kind="ExternalInput")
    dr["y"] = nc.dram_tensor("y", [TQ, C], BF16, kind="ExternalOutput")
    with tile.TileContext(nc) as tc:
        _emit(nc, tc, dr, NK, use_lnw, use_boe)
    nc.compile()
    return nc


def _chunk(a):
    """[C, N] -> [128, C//128, N] with [p, c, n] = a[128c+p, n]."""
    return np.ascontiguousarray(
        a.reshape(NC, P, -1).transpose(1, 0, 2)
    )


def _prep_inputs(inputs):
    f32 = np.float32
    Wq = np.asarray(inputs["Wq"], f32)
    Wk = np.asarray(inputs["Wk"], f32)
    Wv = np.asarray(inputs["Wv"], f32)
    Wo = np.asarray(inputs["Wo"], f32)
    x = np.asarray(inputs["x"], f32)
    mask = np.asarray(inputs["attn_mask"]).reshape(B, T)
    # sort keys so unmasked come first; masked tail chunks are dropped
    perms = [np.argsort(-mask[b], kind="stable") for b in range(B)]
    m1max = max(int(mask[b].sum()) for b in range(B))
    NK = min(NT, max(1, -(-m1max // P)))
    KL = NK * P
    bq = np.asarray(inputs["bq"], f32)
    bk = np.asarray(inputs["bk"], f32)
    bv = np.asarray(inputs["bv"], f32)
    bo = np.asarray(inputs["bo"], f32)
    ln_g = np.asarray(inputs["ln_g"], f32)
    ln_b = np.asarray(inputs["ln_b"], f32)

    boe = (bv @ Wo + bo).astype(f32)
    use_boe = bool(np.any(boe != 0.0))
    use_lnw = bool(np.any(ln_g != 1.0) or np.any(ln_b != 0.0))

    def _fp8(a):
        return np.clip(a, -240.0, 240.0).astype(NPFP8)

    def _proj(a):
        return _fp8(a * WSCALE) if QK_FP8 else a.astype(NPBF16)

    def _projx(a):
        return _fp8(a) if QK_FP8 else a.astype(NPBF16)

    shared = {
        "Wq": _proj(_chunk(Wq)),
        "Wk": _proj(_chunk(Wk)),
        "Wv": _proj(
            np.ascontiguousarray(
                _chunk(Wv).reshape(P, NC, 2, TQ).transpose(0, 2, 1, 3)
            )
        ),
        "Wo": _fp8(_chunk(Wo) * WSCALE)
        if WO_FP8
        else _chunk(Wo).astype(NPBF16),
    }
    if use_boe:
        bsc = (WSCALE * OSCALE) if WO_FP8 else 1.0
        shared["boe"] = (boe * bsc).reshape(1, C).astype(NPBF16)
    if use_lnw:
        shared["lng"] = ln_g.reshape(1, C).astype(f32)
        shared["lnb"] = ln_b.reshape(1, C).astype(f32)
    in_maps = []
    for core in range(8):
        b, half = core // 2, core % 2
        xt = np.ascontiguousarray(x[b].T)  # [C, T]
        pk = perms[b][:KL]
        mfp = mask[b][pk].astype(f32)     # permuted/truncated key mask
        vcol = np.zeros((P, NC), f32)
        vcol[:, :NK] = mfp.reshape(NK, P).T / PSCALE
        vecs = np.stack(
            [
                bq.reshape(NC, P).T * PSCALE,
                bk.reshape(NC, P).T * PSCALE,
                vcol,
            ],
            axis=-1,
        )
        m = dict(shared)
        m["xT"] = _projx(_chunk(np.ascontiguousarray(xt[:, pk])))
        m["xTq"] = _projx(_chunk(xt[:, half * TQ : (half + 1) * TQ]))
        m["vecs"] = np.ascontiguousarray(vecs, f32)
        m["maskv"] = np.ascontiguousarray(mfp.reshape(NK, P).T.astype(NPBF16))
        in_maps.append(m)
    return NK, use_lnw, use_boe, in_maps


def kernel(**inputs):
    global LAST_RESULTS
    NK, use_lnw, use_boe, in_maps = _prep_inputs(inputs)
    key = (
        "nc", NK, use_lnw, use_boe,
        os.environ.get("BASS_NWARM", "12"), WO_FP8, QK_FP8,
    )
    if key not in _CACHE:
        _CACHE[key] = _build(NK=NK, use_lnw=use_lnw, use_boe=use_boe)
    nc = _CACHE[key]

    trace = os.environ.get("KERNEL_TRACE", "0") == "1"
    if trace:
        _ensure_ntff_hook()
    LAST_RESULTS = run_bass_kernel_spmd(
        nc, in_maps, core_ids=list(range(8)), trace=trace
    )
    out = np.empty((B, T, C), np.float32)
    for core in range(8):
        b, half = core // 2, core % 2
        out[b, half * TQ : (half + 1) * TQ, :] = np.asarray(
            LAST_RESULTS.results[core]["y"], dtype=np.float32
        )
    return out


# revision 64
# speedup vs baseline: 1.0624x; 1.0016x over previous
"""Trainium2 Bass kernel for nn_MultiHeadAttention (B=4,T=1024,C=1024,H=16).

Sharding: 8 cores = 4 batches x 2 query-halves. Each core computes, for its
batch b and its 512 query rows:
  V projection (natural layout, mask folded in, +mask column for denominator),
  then per head-pair: Q^T/K^T projection chunks, S^T = K^T.T @ Q^T (row-tiled
  head pairs, D=64 contraction), one exp ACT per key chunk over the merged
  two-head S psum, O^T+denominator via one augmented matmul lhsT=[V_h*m | m],
  normalize via reciprocal + DRAM-bounce partition-broadcast; finally
  Y = O^T.T @ Wo with LN stats read straight from PSUM and the LN apply on
  ACT. Keys are mask-sorted host-side so masked tail chunks drop (NK=5 of 8).

Perf notes (HW ~152us baseline -> ~140-142us this version, cool chip;
sustained back-to-back runs can thermally throttle the PE to ~165us):
  - PE HAM clock gate: the PE runs at 1.2GHz until ~3.4us of sustained
    activity and re-throttles after idle windows, so idle gaps cost double.
    NWARM warmup matmuls on a zeroed tile hold the clock gate open while
    the first input DMAs stream (first real matmul ~8us vs ~16us before).
  - each pair's O^T matmuls are emitted as the NEXT pair's fill work
    (after its QK fill): they depend only on already-finished exps, so
    they are always-ready cover for the serial per-chunk exp chain.
  - Wo-projection chain qc0 (on the freed psP bufs) is emitted as pair 7's
    fill work; qc1 (po bufs), qc2 (s01 bufs), qc3 (po bufs again) follow
    right after the pair loop, each on disjoint psum banks so no chain
    waits on another's LN apply; all four finishers run at the end where
    pair 7's normalize has certainly drained.
  - only sync+scalar have fast (~105 GB/s each) HW DMA queues; V-projection
    inputs land kc-interleaved across both queues in first-use order so
    the first chains trickle along with the DMA instead of stalling on a
    full half; the remaining tensors stream as chunk-halves.
  - fp8/DoubleRow paths exist behind BASS_QK_FP8/BASS_WO_FP8 but are OFF:
    e4m3 quantization injects ~2-4% relative noise into every dot product
    (random-sign sums keep per-element relative error), busting the 2e-2
    gate (measured 4.3e-2 with fp8 projections).
Host gathers the 8 [512,1024] outputs into [4,1024,1024].
"""

import os
import sys

import numpy as np

for _p in ("/opt/trn_rl_repo", "/root/.axon_site/_ro/trn_rl_repo"):
    if os.path.isdir(_p) and _p not in sys.path:
        sys.path.append(_p)

import ml_dtypes  # noqa: E402
import concourse.bass as bass  # noqa: E402
import concourse.mybir as mybir  # noqa: E402
import concourse.tile as tile  # noqa: E402
from concourse import bacc  # noqa: E402
from concourse.bass_utils import run_bass_kernel_spmd  # noqa: E402

BF16 = mybir.dt.bfloat16
FP8 = mybir.dt.float8e4
F32 = mybir.dt.float32
NPBF16 = ml_dtypes.bfloat16
NPFP8 = ml_dtypes.float8_e4m3fn
DR = mybir.MatmulPerfMode.DoubleRow
WSCALE = 64.0     # weights stored x64 in fp8 (keeps them out of subnormals)
WO_FP8 = os.environ.get("BASS_WO_FP8", "0") == "1"
QK_FP8 = os.environ.get("BASS_QK_FP8", "0") == "1"
PROJ_T = FP8 if QK_FP8 else BF16
PSCALE = WSCALE if QK_FP8 else 1.0
OSCALE = 16.0 if WO_FP8 else 1.0  # normalized O^T stored x16 in fp8 (LN absorbs it)
OT_T = FP8 if WO_FP8 else BF16

B, T, C, H = 4, 1024, 1024, 16
D = C // H          # 64
P = 128             # partitions
NC = C // P         # 8 chunks of C
NT = T // P         # 8 chunks of T
TQ = T // 2         # 512 query rows per core
NQ = TQ // P        # 4 query chunks
NPAIR = H // 2      # 8 head pairs
EPS = 1e-5

_CACHE = {}
LAST_RESULTS = None


def _ensure_ntff_hook():
    """Register the axon NTFF profiling hook if the image's antenv lacks it."""
    try:
        import antenv.axon_hooks  # noqa: F401
        return
    except ImportError:
        pass
    try:
        import types

        import antenv
        from trn_agent_boot.trn_boot import _ntff_profile_via_ctypes

        mod = types.ModuleType("antenv.axon_hooks")
        state = {"hook": None}
        mod.set_axon_ntff_profile_hook = lambda h: state.__setitem__("hook", h)
        mod.get_axon_ntff_profile_hook = lambda: state["hook"]
        sys.modules["antenv.axon_hooks"] = mod
        antenv.axon_hooks = mod
        hook = _ntff_profile_via_ctypes("/opt/axon/libaxon_pjrt.so")
        if hook is not None:
            mod.set_axon_ntff_profile_hook(hook)
    except Exception:
        pass


def _emit(nc, tc, dr, NK, use_lnw, use_boe):
    """Emit the per-core Tile program (projections interleaved with attention)."""
    from contextlib import ExitStack

    AF = mybir.ActivationFunctionType
    OP = mybir.AluOpType
    NWARM = int(os.environ.get("BASS_NWARM", "12"))

    with ExitStack() as ctx:
        consts = ctx.enter_context(tc.tile_pool(name="consts", bufs=1))

        # ---- persistent SBUF tiles ----
        KL = NK * P
        VA = [
            consts.tile([P, H, D + 1], BF16, tag=f"va{j}", name=f"va{j}")
            for j in range(NK)
        ]
        # OT pair tiles: OTp[j][:, m, :] = head-pair 2j+m's normalized O^T
        OTp = [
            consts.tile([P, 2, TQ], OT_T, tag=f"otp{j}", name=f"otp{j}")
            for j in range(NPAIR // 2)
        ]
        Wo_sb = consts.tile([P, NC, C], OT_T)
        vecs = consts.tile([P, NC, 3], F32)        # bq | bk | maskf
        maskv = consts.tile([P, NK], BF16)
        eps_t = consts.tile([P, 1], F32)
        warm = consts.tile([P, TQ], BF16)
        if use_lnw:
            lng_rep = consts.tile([P, C], F32)
            lnb_rep = consts.tile([P, C], F32)
        if use_boe:
            boe_sb = consts.tile([1, C], BF16)     # bv@Wo+bo row (partition 0)
            ones_sb = consts.tile([1, P], BF16)    # ones row for bias preload
            nc.vector.memset(ones_sb, 1.0)

        nc.vector.memset(eps_t, EPS)
        nc.vector.memset(warm, 0.0)

        with (
            tc.tile_pool(name="pa", bufs=1) as pa,
            tc.tile_pool(name="pb", bufs=2) as pb,
            tc.tile_pool(name="pbd", bufs=2, space="DRAM") as pbd,
            tc.tile_pool(name="psP", bufs=2, space="PSUM") as psP,
            tc.tile_pool(name="psS", bufs=2, space="PSUM") as psS,
            tc.tile_pool(name="psO", bufs=2, space="PSUM") as psO,
        ):
            # ---- PE warmup: hold the HAM clock gate open while the input
            # DMAs stream; results are never read.
            wps = psS.tile([P, 2, TQ], F32, tag="s01", bufs=1, name="warmps")
            for _ in range(NWARM):
                nc.tensor.matmul(
                    wps[:, 0, :], warm[:, 0:P], warm[:], start=True, stop=True
                )

            xT = pa.tile([P, NC, KL], PROJ_T)
            xTq = pa.tile([P, NC, TQ], PROJ_T)
            Wq_sb = pa.tile([P, NC, C], PROJ_T)
            Wk_sb = pa.tile([P, NC, C], PROJ_T)
            Wv_sb = pa.tile([P, 2, NC, TQ], PROJ_T)  # nn-major

            # ---- input DMAs. Only sync and scalar have fast HW queues;
            # gpsimd's software queue only carries the small vectors. Each
            # tensor is split into two chunk-halves, one per queue, ordered
            # by first use: V-projection inputs first, Wo last.
            HNC = NC // 2
            xT_a = dr["xT"].ap()
            wv0_a = dr["Wv"].ap()[:, 0]
            wv1_a = dr["Wv"].ap()[:, 1]
            # V-projection inputs land kc-interleaved so the first chains
            # trickle along with the DMA instead of stalling on a full half.
            for eng, lo in ((nc.sync, 0), (nc.scalar, 4)):
                eng.dma_start(out=xT[:, lo : lo + 2], in_=xT_a[:, lo : lo + 2])
                eng.dma_start(
                    out=Wv_sb[:, 0, lo : lo + 2], in_=wv0_a[:, lo : lo + 2]
                )
                eng.dma_start(out=xT[:, lo + 2 : lo + 4], in_=xT_a[:, lo + 2 : lo + 4])
                eng.dma_start(
                    out=Wv_sb[:, 0, lo + 2 : lo + 4], in_=wv0_a[:, lo + 2 : lo + 4]
                )
                eng.dma_start(
                    out=Wv_sb[:, 1, lo : lo + 4], in_=wv1_a[:, lo : lo + 4]
                )
            halves = [
                ("xTq", xTq, xTq),
                ("Wq", Wq_sb, Wq_sb), ("Wk", Wk_sb, Wk_sb),
                ("Wo", Wo_sb, Wo_sb),
            ]
            for name, tl, _ in halves:
                a = dr[name].ap()
                nc.sync.dma_start(out=tl[:, 0:HNC], in_=a[:, 0:HNC])
                nc.scalar.dma_start(out=tl[:, HNC:], in_=a[:, HNC:])
            nc.gpsimd.dma_start(out=vecs[:], in_=dr["vecs"].ap()[:])
            nc.gpsimd.dma_start(out=maskv[:], in_=dr["maskv"].ap()[:])
            if use_boe:
                nc.gpsimd.dma_start(out=boe_sb[:], in_=dr["boe"].ap()[:])
            if use_lnw:
                for name, rep in (("lng", lng_rep), ("lnb", lnb_rep)):
                    a = dr[name].ap()
                    bcast = bass.AP(
                        tensor=a.tensor, offset=a.offset, ap=[[0, P], [1, C]]
                    )
                    nc.gpsimd.dma_start(out=rep[:], in_=bcast)

            # ---- V projection: natural [KL, C], masked rows, + mask col ----
            # nn-outer so the first 5 groups only need Wv's nn=0 columns.
            for nn in range(2):
                for tcn in range(NK):
                    ps = psP.tile([P, TQ], F32, tag="psp")
                    if QK_FP8:
                        for kc in range(0, NC, 2):
                            nc.tensor.matmul(
                                ps[:],
                                xT[:, kc : kc + 2, tcn * P : (tcn + 1) * P],
                                Wv_sb[:, nn, kc : kc + 2, :],
                                start=(kc == 0),
                                stop=(kc == NC - 2),
                                perf_mode=DR,
                            )
                    else:
                        for kc in range(NC):
                            nc.tensor.matmul(
                                ps[:],
                                xT[:, kc, tcn * P : (tcn + 1) * P],
                                Wv_sb[:, nn, kc, :],
                                start=(kc == 0),
                                stop=(kc == NC - 1),
                            )
                    nc.vector.tensor_scalar_mul(
                        VA[tcn][:, nn * 8 : (nn + 1) * 8, 0:D],
                        ps[:].rearrange("p (h d) -> p h d", h=8),
                        vecs[:, tcn, 2:3],
                    )
            for tcn in range(NK):
                nc.vector.tensor_copy(
                    out=VA[tcn][:, :, D : D + 1],
                    in_=maskv[:, tcn, None].to_broadcast((P, H, 1)),
                )

            # ---- per head-pair: QT/KT projection, S^T, exp, O^T, normalize.
            # The PE executes matmuls strictly in program order, so pair c+1's
            # Q/K projection matmuls are interleaved into pair c's S/exp phase
            # as fill work; pair 7 gets Wo-projection chains instead.
            def emit_qk(c):
                """Allocate pair c's QT/KT tiles; return (QTc, KTc, steps)."""
                QTc = pb.tile([P, TQ], BF16, tag="qtc", name=f"qt{c}")
                KTc = pb.tile([P, KL], BF16, tag="ktc", name=f"kt{c}")
                steps = []
                psq = psP.tile([P, TQ], F32, tag="psp", name=f"psq{c}")
                if QK_FP8:
                    for kc in range(0, NC, 2):
                        steps.append(
                            lambda kc=kc: nc.tensor.matmul(
                                psq[:],
                                Wq_sb[:, kc : kc + 2, c * P : (c + 1) * P],
                                xTq[:, kc : kc + 2, :],
                                start=(kc == 0),
                                stop=(kc == NC - 2),
                                perf_mode=DR,
                            )
                        )
                else:
                    for kc in range(NC):
                        steps.append(
                            lambda kc=kc: nc.tensor.matmul(
                                psq[:],
                                Wq_sb[:, kc, c * P : (c + 1) * P],
                                xTq[:, kc, :],
                                start=(kc == 0),
                                stop=(kc == NC - 1),
                            )
                        )
                steps.append(
                    lambda: nc.vector.tensor_scalar(
                        QTc[:], psq[:], vecs[:, c, 0:1], 1.0 / PSCALE,
                        OP.add, OP.mult,
                    )
                )
                for ko in range(0, KL, TQ):
                    w = min(TQ, KL - ko)
                    psk = psP.tile([P, TQ], F32, tag="psp", name=f"psk{c}{ko}")
                    if QK_FP8:
                        for kc in range(0, NC, 2):
                            steps.append(
                                lambda kc=kc, ko=ko, w=w, psk=psk: nc.tensor.matmul(
                                    psk[:, :w],
                                    Wk_sb[:, kc : kc + 2, c * P : (c + 1) * P],
                                    xT[:, kc : kc + 2, ko : ko + w],
                                    start=(kc == 0),
                                    stop=(kc == NC - 2),
                                    perf_mode=DR,
                                )
                            )
                    else:
                        for kc in range(NC):
                            steps.append(
                                lambda kc=kc, ko=ko, w=w, psk=psk: nc.tensor.matmul(
                                    psk[:, :w],
                                    Wk_sb[:, kc, c * P : (c + 1) * P],
                                    xT[:, kc, ko : ko + w],
                                    start=(kc == 0),
                                    stop=(kc == NC - 1),
                                )
                            )
                    steps.append(
                        lambda ko=ko, w=w, psk=psk: nc.vector.tensor_scalar(
                            KTc[:, ko : ko + w], psk[:, :w], vecs[:, c, 1:2],
                            1.0 / PSCALE, OP.add, OP.mult,
                        )
                    )
                return QTc, KTc, steps

            # ---- Wo chain machinery (chains scheduled as late-pair fill) ----
            def wo_make(qc, pse):
                """pse = [psum AP nn0, psum AP nn1]. Returns (mm, stat, finish)."""
                qs = slice(qc * P, (qc + 1) * P)
                stats = pb.tile(
                    [P, 2, nc.vector.BN_STATS_DIM], F32, tag="stats",
                    name=f"stats{qc}",
                )

                def mm(nn, mc):
                    # mc in {0, 2, 4}: DoubleRow over the OT pair tile;
                    # mc in {6, 7}: plain fp8 (keeps the last two OTs
                    # independently schedulable).
                    def f():
                        if mc == 0 and use_boe:
                            nc.tensor.matmul(
                                pse[nn],
                                ones_sb[:, 0:P],
                                boe_sb[:, nn * TQ : (nn + 1) * TQ],
                                start=True, stop=False,
                            )
                        if mc < 6 and WO_FP8:
                            nc.tensor.matmul(
                                pse[nn],
                                OTp[mc // 2][:, :, qs],
                                Wo_sb[:, mc : mc + 2, nn * TQ : (nn + 1) * TQ],
                                start=(mc == 0 and not use_boe),
                                stop=False,
                                perf_mode=DR,
                            )
                        elif mc < 6:
                            for m2 in (mc, mc + 1):
                                nc.tensor.matmul(
                                    pse[nn],
                                    OTp[m2 // 2][:, m2 % 2, qs],
                                    Wo_sb[:, m2, nn * TQ : (nn + 1) * TQ],
                                    start=(m2 == 0 and not use_boe),
                                    stop=False,
                                )
                        else:
                            nc.tensor.matmul(
                                pse[nn],
                                OTp[mc // 2][:, mc % 2, qs],
                                Wo_sb[:, mc, nn * TQ : (nn + 1) * TQ],
                                start=False,
                                stop=(mc == NC - 1),
                            )
                    return f

                def stat(nn):
                    def f():
                        nc.vector.bn_stats(out=stats[:, nn, :], in_=pse[nn])
                    return f

                def finish():
                    qsl = slice(qc * P, (qc + 1) * P)
                    mv = pb.tile([P, nc.vector.BN_AGGR_DIM], F32, tag="mv")
                    nc.vector.bn_aggr(out=mv[:], in_=stats[:])
                    rstd = pb.tile([P, 1], F32, tag="rstd")
                    nmr = pb.tile([P, 1], F32, tag="nmr")
                    nc.scalar.activation(
                        out=rstd[:], in_=mv[:, 1:2],
                        func=AF.Sqrt, bias=eps_t[:], scale=1.0,
                    )
                    nc.vector.reciprocal(out=rstd[:], in_=rstd[:])
                    nc.vector.tensor_scalar(
                        nmr[:], mv[:, 0:1], rstd[:], -1.0,
                        OP.mult, OP.mult,
                    )
                    Y = pb.tile([P, C], BF16, tag="ysb", bufs=2)
                    for nn in range(2):
                        cs = slice(nn * TQ, (nn + 1) * TQ)
                        if nn == 0:
                            nc.scalar.activation(
                                out=Y[:, cs], in_=pse[nn],
                                func=AF.Identity,
                                bias=nmr[:], scale=rstd[:],
                            )
                        else:
                            nc.vector.scalar_tensor_tensor(
                                Y[:, cs], pse[nn], rstd[:],
                                nmr[:, 0:1].to_broadcast((P, TQ)),
                                OP.mult, OP.add,
                            )
                        if use_lnw:
                            nc.vector.tensor_tensor(
                                Y[:, cs], Y[:, cs], lng_rep[:, cs], OP.mult
                            )
                            nc.gpsimd.tensor_tensor(
                                Y[:, cs], Y[:, cs], lnb_rep[:, cs], OP.add
                            )
                        eng = nc.sync if (2 * qc + nn) % 2 == 0 else nc.scalar
                        eng.dma_start(out=dr["y"].ap()[qsl, cs], in_=Y[:, cs])

                return mm, stat, finish

            wo = {}  # qc -> (mm, stat, finish)

            def make_osteps(po0, po1, ets, g0, g1):
                steps = []
                for jc in range(NK):
                    steps.append(
                        lambda jc=jc: nc.tensor.matmul(
                            po0[0 : D + 1, :], VA[jc][:, g0, :],
                            ets[jc][:, 0, :],
                            start=(jc == 0), stop=(jc == NK - 1),
                        )
                    )
                for jc in range(NK):
                    steps.append(
                        lambda jc=jc: nc.tensor.matmul(
                            po1[0 : D + 1, :], VA[jc][:, g1, :],
                            ets[jc][:, 1, :],
                            start=(jc == 0), stop=(jc == NK - 1),
                        )
                    )
                return steps

            def norm(c, po0, po1):
                # d rows live on psum partition 64: copy out, DMA-shift to
                # partition 0 (approx-recip ucode is broken at base!=0),
                # reciprocal, then DRAM-bounce partition broadcast.
                dsb = pb.tile([P, 2 * TQ], F32, tag="dsb")
                dp0 = pb.tile([1, 2 * TQ], F32, tag="dp0")
                rp0 = pb.tile([1, 2 * TQ], F32, tag="rp0")
                rrep = pb.tile([D, 2 * TQ], F32, tag="rrep")
                nc.vector.tensor_scalar_mul(
                    dsb[D : D + 1, 0:TQ], po0[D : D + 1, :], 1.0 / OSCALE
                )
                nc.vector.tensor_scalar_mul(
                    dsb[D : D + 1, TQ:], po1[D : D + 1, :], 1.0 / OSCALE
                )
                nc.sync.dma_start(out=dp0[0:1, :], in_=dsb[D : D + 1, :])
                nc.vector.reciprocal_approx_fast(out=rp0[:], in_=dp0[:])
                rdram = pbd.tile([1, 2 * TQ], F32, tag="rdram")
                nc.sync.dma_start(out=rdram[:], in_=rp0[0:1, :])
                src = rdram[0:1, :]
                bcast = bass.AP(
                    tensor=src.tensor, offset=src.offset, ap=[[0, D]] + src.ap[1:]
                )
                nc.sync.dma_start(out=rrep[:], in_=bcast)
                # normalize: even head straight into OT, odd staged + DMA shift
                odd = pb.tile([D, TQ], OT_T, tag="odd")
                nc.vector.tensor_tensor(
                    OTp[c // 2][0:D, c % 2, :], po0[0:D, :], rrep[:, 0:TQ], OP.mult
                )
                nc.vector.tensor_tensor(
                    odd[:], po1[0:D, :], rrep[:, TQ:], OP.mult
                )
                nc.sync.dma_start(out=OTp[c // 2][D:P, c % 2, :], in_=odd[:])

            QTc, KTc, steps = emit_qk(0)
            for st in steps:
                st()
            prev = None  # (c-1, po0, po1) awaiting normalize
            for c in range(NPAIR):
                h0, h1 = 2 * c, 2 * c + 1
                if c + 1 < NPAIR:
                    nQT, nKT, nsteps = emit_qk(c + 1)
                else:
                    nQT, nKT = None, None
                    # pair 7's extra fill = Wo chain qc0 (freed psP bufs)
                    pw0 = psP.tile([P, TQ], F32, tag="psp", name="wo0a")
                    pw1 = psP.tile([P, TQ], F32, tag="psp", name="wo0b")
                    wo[0] = wo_make(0, [pw0[:], pw1[:]])
                    nsteps = [wo[0][0](nn, mc) for nn in range(2) for mc in (0, 2, 4)]

                # this pair's O psums; its matmuls run as the NEXT pair's
                # fill (they depend only on already-finished exps, so they
                # are always-ready work during the serial exp chain).
                po0 = psO.tile([P, TQ], F32, tag="po0", bufs=2)
                po1 = psO.tile([P, TQ], F32, tag="po1", bufs=2)
                ets = [
                    pb.tile([P, 2, TQ], BF16, tag=f"et{jc}", bufs=2, name=f"et{jc}")
                    for jc in range(NK)
                ]
                osteps = make_osteps(po0, po1, ets, h0, h1)
                if c + 1 < NPAIR:
                    # QK fill first (its results gate the next pair's S);
                    # the previous pair's O steps absorb the late slots.
                    if prev is not None:
                        nsteps = nsteps + prev[3]
                else:
                    if prev is not None:
                        nsteps = prev[3] + nsteps
                    nsteps = nsteps + osteps

                nfill = len(nsteps)
                cuts = [0]
                for jc in range(NK):
                    frac = 0 if jc == 0 else jc / (NK - 1)
                    cuts.append(round(nfill * frac))
                for jc in range(NK):
                    js = slice(jc * P, (jc + 1) * P)
                    s01 = psS.tile([P, 2, TQ], F32, tag="s01", bufs=1)
                    nc.tensor.matmul(
                        s01[:, 0, :],
                        KTc[0:D, js],
                        QTc[0:D, :],
                        start=True, stop=True,
                        tile_position=(0, 0),
                    )
                    nc.tensor.matmul(
                        s01[:, 1, :],
                        KTc[D:P, js],
                        QTc[D:P, :],
                        start=True, stop=True,
                        tile_position=(D, 0),
                    )
                    nc.scalar.activation(
                        out=ets[jc][:], in_=s01[:],
                        func=AF.Exp, scale=0.125,
                    )
                    for st in nsteps[cuts[jc] : cuts[jc + 1]]:
                        st()
                for st in nsteps[cuts[NK] :]:
                    st()

                if prev is not None:
                    norm(prev[0], prev[1], prev[2])
                prev = (c, po0, po1, osteps)
                QTc, KTc = nQT, nKT
            norm(prev[0], prev[1], prev[2])

            # ---- remaining output-projection chains + LN. qc2 takes the
            # s01 bufs (free after pair 7's exps); qc3 reuses qc0's psP
            # bufs once qc0's LN apply has drained them. Finishers are
            # interleaved so the applies overlap later chains' matmuls.
            pw1a = psO.tile([P, TQ], F32, tag="po0", bufs=2, name="wo1a")
            pw1b = psO.tile([P, TQ], F32, tag="po1", bufs=2, name="wo1b")
            wo[1] = wo_make(1, [pw1a[:], pw1b[:]])
            for nn in range(2):
                for mc in (0, 2, 4):
                    wo[1][0](nn, mc)()
            for q in (0, 1):
                for nn in range(2):
                    wo[q][0](nn, 6)()
            sw2 = psS.tile([P, 2, TQ], F32, tag="s01", bufs=1, name="wo2")
            wo[2] = wo_make(2, [sw2[:, 0, :], sw2[:, 1, :]])
            for nn in range(2):
                for mc in (0, 2, 4, 6):
                    wo[2][0](nn, mc)()
            pw3a = psO.tile([P, TQ], F32, tag="po0", bufs=2, name="wo3a")
            pw3b = psO.tile([P, TQ], F32, tag="po1", bufs=2, name="wo3b")
            wo[3] = wo_make(3, [pw3a[:], pw3b[:]])
            for nn in range(2):
                for mc in (0, 2, 4, 6):
                    wo[3][0](nn, mc)()
            # finishers: last accumulation (needs pair 7's OT), stats, LN
            for q in (0, 1, 2, 3):
                mm, stat, finish = wo[q]
                for nn in range(2):
                    mm(nn, 7)()
                    stat(nn)()
                finish()


def _build(NK=NT, use_lnw=True, use_boe=True):
    nc = bacc.Bacc("TRN2", target_bir_lowering=False, debug=False, num_devices=8)
    dr = {}
    dr["xT"] = nc.dram_tensor("xT", [P, NC, NK * P], PROJ_T, kind="ExternalInput")
    dr["xTq"] = nc.dram_tensor("xTq", [P, NC, TQ], PROJ_T, kind="ExternalInput")
    for w in ("Wq", "Wk"):
        dr[w] = nc.dram_tensor(w, [P, NC, C], PROJ_T, kind="ExternalInput")
    dr["Wo"] = nc.dram_tensor("Wo", [P, NC, C], OT_T, kind="ExternalInput")
    dr["Wv"] = nc.dram_tensor("Wv", [P, 2, NC, TQ], PROJ_T, kind="ExternalInput")
    dr["vecs"] = nc.dram_tensor("vecs", [P, NC, 3], F32, kind="ExternalInput")
    dr["maskv"] = nc.dram_tensor("maskv", [P, NK], BF16, kind="ExternalInput")
    if use_boe:
        dr["boe"] = nc.dram_tensor("boe", [1, C], BF16, kind="ExternalInput")
    if use_lnw:
        for v in ("lng", "lnb"):
            dr[v] = nc.dram_tensor(v, [1, C], F32, kind="ExternalInput")
    dr["y"] = nc.dram_tensor("y", [TQ, C], BF16, kind="ExternalOutput")
    with tile.TileContext(nc) as tc:
        _emit(nc, tc, dr, NK, use_lnw, use_boe)
    nc.compile()
    return nc


def _chunk(a):
    """[C, N] -> [128, C//128, N] with [p, c, n] = a[128c+p, n]."""
    return np.ascontiguousarray(
        a.reshape(NC, P, -1).transpose(1, 0, 2)
    )


def _prep_inputs(inputs):
    f32 = np.float32
    Wq = np.asarray(inputs["Wq"], f32)
    Wk = np.asarray(inputs["Wk"], f32)
    Wv = np.asarray(inputs["Wv"], f32)
    Wo = np.asarray(inputs["Wo"], f32)
    x = np.asarray(inputs["x"], f32)
    mask = np.asarray(inputs["attn_mask"]).reshape(B, T)
    # sort keys so unmasked come first; masked tail chunks are dropped
    perms = [np.argsort(-mask[b], kind="stable") for b in range(B)]
    m1max = max(int(mask[b].sum()) for b in range(B))
    NK = min(NT, max(1, -(-m1max // P)))
    KL = NK * P
    bq = np.asarray(inputs["bq"], f32)
    bk = np.asarray(inputs["bk"], f32)
    bv = np.asarray(inputs["bv"], f32)
    bo = np.asarray(inputs["bo"], f32)
    ln_g = np.asarray(inputs["ln_g"], f32)
    ln_b = np.asarray(inputs["ln_b"], f32)

    boe = (bv @ Wo + bo).astype(f32)
    use_boe = bool(np.any(boe != 0.0))
    use_lnw = bool(np.any(ln_g != 1.0) or np.any(ln_b != 0.0))

    def _fp8(a):
        return np.clip(a, -240.0, 240.0).astype(NPFP8)

    def _proj(a):
        return _fp8(a * WSCALE) if QK_FP8 else a.astype(NPBF16)

    def _projx(a):
        return _fp8(a) if QK_FP8 else a.astype(NPBF16)

    shared = {
        "Wq": _proj(_chunk(Wq)),
        "Wk": _proj(_chunk(Wk)),
        "Wv": _proj(
            np.ascontiguousarray(
                _chunk(Wv).reshape(P, NC, 2, TQ).transpose(0, 2, 1, 3)
            )
        ),
        "Wo": _fp8(_chunk(Wo) * WSCALE)
        if WO_FP8
        else _chunk(Wo).astype(NPBF16),
    }
    if use_boe:
        bsc = (WSCALE * OSCALE) if WO_FP8 else 1.0
        shared["boe"] = (boe * bsc).reshape(1, C).astype(NPBF16)
    if use_lnw:
        shared["lng"] = ln_g.reshape(1, C).astype(f32)
        shared["lnb"] = ln_b.reshape(1, C).astype(f32)
    in_maps = []
    for core in range(8):
        b, half = core // 2, core % 2
        xt = np.ascontiguousarray(x[b].T)  # [C, T]
        pk = perms[b][:KL]
        mfp = mask[b][pk].astype(f32)     # permuted/truncated key mask
        vcol = np.zeros((P, NC), f32)
        vcol[:, :NK] = mfp.reshape(NK, P).T / PSCALE
        vecs = np.stack(
            [
                bq.reshape(NC, P).T * PSCALE,
                bk.reshape(NC, P).T * PSCALE,
                vcol,
            ],
            axis=-1,
        )
        m = dict(shared)
        m["xT"] = _projx(_chunk(np.ascontiguousarray(xt[:, pk])))
        m["xTq"] = _projx(_chunk(xt[:, half * TQ : (half + 1) * TQ]))
        m["vecs"] = np.ascontiguousarray(vecs, f32)
        m["maskv"] = np.ascontiguousarray(mfp.reshape(NK, P).T.astype(NPBF16))
        in_maps.append(m)
    return NK, use_lnw, use_boe, in_maps


def kernel(**inputs):
    global LAST_RESULTS
    NK, use_lnw, use_boe, in_maps = _prep_inputs(inputs)
    key = (
        "nc", NK, use_lnw, use_boe,
        os.environ.get("BASS_NWARM", "12"), WO_FP8, QK_FP8,
    )
    if key not in _CACHE:
        _CACHE[key] = _build(NK=NK, use_lnw=use_lnw, use_boe=use_boe)
    nc = _CACHE[key]

    trace = os.environ.get("KERNEL_TRACE", "0") == "1"
    if trace:
        _ensure_ntff_hook()
    LAST_RESULTS = run_bass_kernel_spmd(
        nc, in_maps, core_ids=list(range(8)), trace=trace
    )
    out = np.empty((B, T, C), np.float32)
    for core in range(8):
        b, half = core // 2, core % 2
        out[b, half * TQ : (half + 1) * TQ, :] = np.asarray(
            LAST_RESULTS.results[core]["y"], dtype=np.float32
        )
    return out


# revision 66
# speedup vs baseline: 1.0755x; 1.0124x over previous
"""Trainium2 Bass kernel for nn_MultiHeadAttention (B=4,T=1024,C=1024,H=16).

Sharding: 8 cores = 4 batches x 2 query-halves. Each core computes, for its
batch b and its 512 query rows:
  V projection (natural layout, mask folded in, +mask column for denominator),
  then per head-pair: Q^T/K^T projection chunks, S^T = K^T.T @ Q^T (row-tiled
  head pairs, D=64 contraction), one exp ACT per key chunk over the merged
  two-head S psum, O^T+denominator via one augmented matmul lhsT=[V_h*m | m],
  normalize via reciprocal + DRAM-bounce partition-broadcast; finally
  Y = O^T.T @ Wo with LN stats read straight from PSUM and the LN apply on
  ACT. Keys are mask-sorted host-side so masked tail chunks drop (NK=5 of 8).

Perf notes (HW ~152us baseline -> ~140-142us this version, cool chip;
sustained back-to-back runs can thermally throttle the PE to ~165us):
  - PE HAM clock gate: the PE runs at 1.2GHz until ~3.4us of sustained
    activity and re-throttles after idle windows, so idle gaps cost double.
    NWARM warmup matmuls on a zeroed tile hold the clock gate open while
    the first input DMAs stream (first real matmul ~8us vs ~16us before).
  - each pair's O^T matmuls are emitted as the NEXT pair's fill work
    (after its QK fill): they depend only on already-finished exps, so
    they are always-ready cover for the serial per-chunk exp chain.
  - Wo-projection chain qc0 (on the freed psP bufs) is emitted as pair 7's
    fill work; qc1 (po bufs), qc2 (s01 bufs), qc3 (po bufs again) follow
    right after the pair loop, each on disjoint psum banks so no chain
    waits on another's LN apply; all four finishers run at the end where
    pair 7's normalize has certainly drained.
  - only sync+scalar have fast (~105 GB/s each) HW DMA queues; V-projection
    inputs land kc-interleaved across both queues in first-use order so
    the first chains trickle along with the DMA instead of stalling on a
    full half; the remaining tensors stream as chunk-halves.
  - fp8/DoubleRow paths exist behind BASS_QK_FP8/BASS_WO_FP8 but are OFF:
    e4m3 quantization injects ~2-4% relative noise into every dot product
    (random-sign sums keep per-element relative error), busting the 2e-2
    gate (measured 4.3e-2 with fp8 projections).
Host gathers the 8 [512,1024] outputs into [4,1024,1024].
"""

import os
import sys

import numpy as np

for _p in ("/opt/trn_rl_repo", "/root/.axon_site/_ro/trn_rl_repo"):
    if os.path.isdir(_p) and _p not in sys.path:
        sys.path.append(_p)

import ml_dtypes  # noqa: E402
import concourse.bass as bass  # noqa: E402
import concourse.mybir as mybir  # noqa: E402
import concourse.tile as tile  # noqa: E402
from concourse import bacc  # noqa: E402
from concourse.bass_utils import run_bass_kernel_spmd  # noqa: E402

BF16 = mybir.dt.bfloat16
FP8 = mybir.dt.float8e4
F32 = mybir.dt.float32
NPBF16 = ml_dtypes.bfloat16
NPFP8 = ml_dtypes.float8_e4m3fn
DR = mybir.MatmulPerfMode.DoubleRow
WSCALE = 64.0     # weights stored x64 in fp8 (keeps them out of subnormals)
WO_FP8 = os.environ.get("BASS_WO_FP8", "0") == "1"
QK_FP8 = os.environ.get("BASS_QK_FP8", "0") == "1"
PROJ_T = FP8 if QK_FP8 else BF16
PSCALE = WSCALE if QK_FP8 else 1.0
OSCALE = 16.0 if WO_FP8 else 1.0  # normalized O^T stored x16 in fp8 (LN absorbs it)
OT_T = FP8 if WO_FP8 else BF16

B, T, C, H = 4, 1024, 1024, 16
D = C // H          # 64
P = 128             # partitions
NC = C // P         # 8 chunks of C
NT = T // P         # 8 chunks of T
TQ = T // 2         # 512 query rows per core
NQ = TQ // P        # 4 query chunks
NPAIR = H // 2      # 8 head pairs
EPS = 1e-5

_CACHE = {}
LAST_RESULTS = None


def _ensure_ntff_hook():
    """Register the axon NTFF profiling hook if the image's antenv lacks it."""
    try:
        import antenv.axon_hooks  # noqa: F401
        return
    except ImportError:
        pass
    try:
        import types

        import antenv
        from trn_agent_boot.trn_boot import _ntff_profile_via_ctypes

        mod = types.ModuleType("antenv.axon_hooks")
        state = {"hook": None}
        mod.set_axon_ntff_profile_hook = lambda h: state.__setitem__("hook", h)
        mod.get_axon_ntff_profile_hook = lambda: state["hook"]
        sys.modules["antenv.axon_hooks"] = mod
        antenv.axon_hooks = mod
        hook = _ntff_profile_via_ctypes("/opt/axon/libaxon_pjrt.so")
        if hook is not None:
            mod.set_axon_ntff_profile_hook(hook)
    except Exception:
        pass


def _emit(nc, tc, dr, NK, use_lnw, use_boe):
    """Emit the per-core Tile program (projections interleaved with attention)."""
    from contextlib import ExitStack

    AF = mybir.ActivationFunctionType
    OP = mybir.AluOpType
    NWARM = int(os.environ.get("BASS_NWARM", "12"))

    with ExitStack() as ctx:
        consts = ctx.enter_context(tc.tile_pool(name="consts", bufs=1))

        # ---- persistent SBUF tiles ----
        KL = NK * P
        VA = [
            consts.tile([P, H, D + 1], BF16, tag=f"va{j}", name=f"va{j}")
            for j in range(NK)
        ]
        # OT pair tiles: OTp[j][:, m, :] = head-pair 2j+m's normalized O^T
        OTp = [
            consts.tile([P, 2, TQ], OT_T, tag=f"otp{j}", name=f"otp{j}")
            for j in range(NPAIR // 2)
        ]
        Wo_sb = consts.tile([P, NC, C], OT_T)
        vecs = consts.tile([P, NC, 3], F32)        # bq | bk | maskf
        maskv = consts.tile([P, NK], BF16)
        eps_t = consts.tile([P, 1], F32)
        warm = consts.tile([P, TQ], BF16)
        if use_lnw:
            lng_rep = consts.tile([P, C], F32)
            lnb_rep = consts.tile([P, C], F32)
        if use_boe:
            boe_sb = consts.tile([1, C], BF16)     # bv@Wo+bo row (partition 0)
            ones_sb = consts.tile([1, P], BF16)    # ones row for bias preload
            nc.vector.memset(ones_sb, 1.0)

        nc.vector.memset(eps_t, EPS)
        nc.vector.memset(warm, 0.0)

        with (
            tc.tile_pool(name="pa", bufs=1) as pa,
            tc.tile_pool(name="pb", bufs=2) as pb,
            tc.tile_pool(name="pbd", bufs=2, space="DRAM") as pbd,
            tc.tile_pool(name="psP", bufs=2, space="PSUM") as psP,
            tc.tile_pool(name="psS", bufs=2, space="PSUM") as psS,
            tc.tile_pool(name="psO", bufs=2, space="PSUM") as psO,
        ):
            # ---- PE warmup: hold the HAM clock gate open while the input
            # DMAs stream; results are never read.
            wps = psS.tile([P, 2, TQ], F32, tag="s01", bufs=1, name="warmps")
            for _ in range(NWARM):
                nc.tensor.matmul(
                    wps[:, 0, :], warm[:, 0:P], warm[:], start=True, stop=True
                )

            xT = pa.tile([P, NC, KL], PROJ_T)
            xTq = pa.tile([P, NC, TQ], PROJ_T)
            Wq_sb = pa.tile([P, NC, C], PROJ_T)
            Wk_sb = pa.tile([P, NC, C], PROJ_T)
            Wv_sb = pa.tile([P, 2, NC, TQ], PROJ_T)  # nn-major

            # ---- input DMAs. Only sync and scalar have fast HW queues;
            # gpsimd's software queue only carries the small vectors. Each
            # tensor is split into two chunk-halves, one per queue, ordered
            # by first use: V-projection inputs first, Wo last.
            HNC = NC // 2
            xT_a = dr["xT"].ap()
            wv0_a = dr["Wv"].ap()[:, 0]
            wv1_a = dr["Wv"].ap()[:, 1]
            # V-projection inputs land kc-interleaved so the first chains
            # trickle along with the DMA instead of stalling on a full half.
            for eng, lo in ((nc.sync, 0), (nc.scalar, 4)):
                eng.dma_start(out=xT[:, lo : lo + 2], in_=xT_a[:, lo : lo + 2])
                eng.dma_start(
                    out=Wv_sb[:, 0, lo : lo + 2], in_=wv0_a[:, lo : lo + 2]
                )
                eng.dma_start(out=xT[:, lo + 2 : lo + 4], in_=xT_a[:, lo + 2 : lo + 4])
                eng.dma_start(
                    out=Wv_sb[:, 0, lo + 2 : lo + 4], in_=wv0_a[:, lo + 2 : lo + 4]
                )
                eng.dma_start(
                    out=Wv_sb[:, 1, lo : lo + 4], in_=wv1_a[:, lo : lo + 4]
                )
            halves = [
                ("xTq", xTq, xTq),
                ("Wq", Wq_sb, Wq_sb), ("Wk", Wk_sb, Wk_sb),
                ("Wo", Wo_sb, Wo_sb),
            ]
            for name, tl, _ in halves:
                a = dr[name].ap()
                nc.sync.dma_start(out=tl[:, 0:HNC], in_=a[:, 0:HNC])
                nc.scalar.dma_start(out=tl[:, HNC:], in_=a[:, HNC:])
            nc.gpsimd.dma_start(out=vecs[:], in_=dr["vecs"].ap()[:])
            nc.gpsimd.dma_start(out=maskv[:], in_=dr["maskv"].ap()[:])
            if use_boe:
                nc.gpsimd.dma_start(out=boe_sb[:], in_=dr["boe"].ap()[:])
            if use_lnw:
                for name, rep in (("lng", lng_rep), ("lnb", lnb_rep)):
                    a = dr[name].ap()
                    bcast = bass.AP(
                        tensor=a.tensor, offset=a.offset, ap=[[0, P], [1, C]]
                    )
                    nc.gpsimd.dma_start(out=rep[:], in_=bcast)

            # ---- V projection: natural [KL, C], masked rows, + mask col ----
            # nn-outer so the first 5 groups only need Wv's nn=0 columns.
            for nn in range(2):
                for tcn in range(NK):
                    ps = psP.tile([P, TQ], F32, tag="psp")
                    if QK_FP8:
                        for kc in range(0, NC, 2):
                            nc.tensor.matmul(
                                ps[:],
                                xT[:, kc : kc + 2, tcn * P : (tcn + 1) * P],
                                Wv_sb[:, nn, kc : kc + 2, :],
                                start=(kc == 0),
                                stop=(kc == NC - 2),
                                perf_mode=DR,
                            )
                    else:
                        for kc in range(NC):
                            nc.tensor.matmul(
                                ps[:],
                                xT[:, kc, tcn * P : (tcn + 1) * P],
                                Wv_sb[:, nn, kc, :],
                                start=(kc == 0),
                                stop=(kc == NC - 1),
                            )
                    nc.vector.tensor_scalar_mul(
                        VA[tcn][:, nn * 8 : (nn + 1) * 8, 0:D],
                        ps[:].rearrange("p (h d) -> p h d", h=8),
                        vecs[:, tcn, 2:3],
                    )
            for tcn in range(NK):
                nc.vector.tensor_copy(
                    out=VA[tcn][:, :, D : D + 1],
                    in_=maskv[:, tcn, None].to_broadcast((P, H, 1)),
                )

            # ---- per head-pair: QT/KT projection, S^T, exp, O^T, normalize.
            # The PE executes matmuls strictly in program order, so pair c+1's
            # Q/K projection matmuls are interleaved into pair c's S/exp phase
            # as fill work; pair 7 gets Wo-projection chains instead.
            def emit_qk(c):
                """Allocate pair c's QT/KT tiles; return (QTc, KTc, steps)."""
                QTc = pb.tile([P, TQ], BF16, tag="qtc", name=f"qt{c}")
                KTc = pb.tile([P, KL], BF16, tag="ktc", name=f"kt{c}")
                steps = []
                psq = psP.tile([P, TQ], F32, tag="psp", name=f"psq{c}")
                if QK_FP8:
                    for kc in range(0, NC, 2):
                        steps.append(
                            lambda kc=kc: nc.tensor.matmul(
                                psq[:],
                                Wq_sb[:, kc : kc + 2, c * P : (c + 1) * P],
                                xTq[:, kc : kc + 2, :],
                                start=(kc == 0),
                                stop=(kc == NC - 2),
                                perf_mode=DR,
                            )
                        )
                else:
                    for kc in range(NC):
                        steps.append(
                            lambda kc=kc: nc.tensor.matmul(
                                psq[:],
                                Wq_sb[:, kc, c * P : (c + 1) * P],
                                xTq[:, kc, :],
                                start=(kc == 0),
                                stop=(kc == NC - 1),
                            )
                        )
                steps.append(
                    lambda: nc.vector.tensor_scalar(
                        QTc[:], psq[:], vecs[:, c, 0:1], 1.0 / PSCALE,
                        OP.add, OP.mult,
                    )
                )
                for ko in range(0, KL, TQ):
                    w = min(TQ, KL - ko)
                    psk = psP.tile([P, TQ], F32, tag="psp", name=f"psk{c}{ko}")
                    if QK_FP8:
                        for kc in range(0, NC, 2):
                            steps.append(
                                lambda kc=kc, ko=ko, w=w, psk=psk: nc.tensor.matmul(
                                    psk[:, :w],
                                    Wk_sb[:, kc : kc + 2, c * P : (c + 1) * P],
                                    xT[:, kc : kc + 2, ko : ko + w],
                                    start=(kc == 0),
                                    stop=(kc == NC - 2),
                                    perf_mode=DR,
                                )
                            )
                    else:
                        for kc in range(NC):
                            steps.append(
                                lambda kc=kc, ko=ko, w=w, psk=psk: nc.tensor.matmul(
                                    psk[:, :w],
                                    Wk_sb[:, kc, c * P : (c + 1) * P],
                                    xT[:, kc, ko : ko + w],
                                    start=(kc == 0),
                                    stop=(kc == NC - 1),
                                )
                            )
                    steps.append(
                        lambda ko=ko, w=w, psk=psk: nc.vector.tensor_scalar(
                            KTc[:, ko : ko + w], psk[:, :w], vecs[:, c, 1:2],
                            1.0 / PSCALE, OP.add, OP.mult,
                        )
                    )
                return QTc, KTc, steps

            # ---- Wo chain machinery (chains scheduled as late-pair fill) ----
            def wo_make(qc, pse):
                """pse = [psum AP nn0, psum AP nn1]. Returns (mm, stat, finish)."""
                qs = slice(qc * P, (qc + 1) * P)
                stats = pb.tile(
                    [P, 2, nc.vector.BN_STATS_DIM], F32, tag="stats",
                    name=f"stats{qc}",
                )

                def mm(nn, mc):
                    # mc in {0, 2, 4}: DoubleRow over the OT pair tile;
                    # mc in {6, 7}: plain fp8 (keeps the last two OTs
                    # independently schedulable).
                    def f():
                        if mc == 0 and use_boe:
                            nc.tensor.matmul(
                                pse[nn],
                                ones_sb[:, 0:P],
                                boe_sb[:, nn * TQ : (nn + 1) * TQ],
                                start=True, stop=False,
                            )
                        if mc < 6 and WO_FP8:
                            nc.tensor.matmul(
                                pse[nn],
                                OTp[mc // 2][:, :, qs],
                                Wo_sb[:, mc : mc + 2, nn * TQ : (nn + 1) * TQ],
                                start=(mc == 0 and not use_boe),
                                stop=False,
                                perf_mode=DR,
                            )
                        elif mc < 6:
                            for m2 in (mc, mc + 1):
                                nc.tensor.matmul(
                                    pse[nn],
                                    OTp[m2 // 2][:, m2 % 2, qs],
                                    Wo_sb[:, m2, nn * TQ : (nn + 1) * TQ],
                                    start=(m2 == 0 and not use_boe),
                                    stop=False,
                                )
                        else:
                            nc.tensor.matmul(
                                pse[nn],
                                OTp[mc // 2][:, mc % 2, qs],
                                Wo_sb[:, mc, nn * TQ : (nn + 1) * TQ],
                                start=False,
                                stop=(mc == NC - 1),
                            )
                    return f

                def stat(nn):
                    def f():
                        nc.vector.bn_stats(out=stats[:, nn, :], in_=pse[nn])
                    return f

                def finish():
                    qsl = slice(qc * P, (qc + 1) * P)
                    mv = pb.tile([P, nc.vector.BN_AGGR_DIM], F32, tag="mv")
                    nc.vector.bn_aggr(out=mv[:], in_=stats[:])
                    rstd = pb.tile([P, 1], F32, tag="rstd")
                    nmr = pb.tile([P, 1], F32, tag="nmr")
                    nc.scalar.activation(
                        out=rstd[:], in_=mv[:, 1:2],
                        func=AF.Sqrt, bias=eps_t[:], scale=1.0,
                    )
                    nc.vector.reciprocal(out=rstd[:], in_=rstd[:])
                    nc.vector.tensor_scalar(
                        nmr[:], mv[:, 0:1], rstd[:], -1.0,
                        OP.mult, OP.mult,
                    )
                    Y = pb.tile([P, C], BF16, tag="ysb", bufs=2)
                    for nn in range(2):
                        cs = slice(nn * TQ, (nn + 1) * TQ)
                        if nn == 0:
                            nc.scalar.activation(
                                out=Y[:, cs], in_=pse[nn],
                                func=AF.Identity,
                                bias=nmr[:], scale=rstd[:],
                            )
                        else:
                            nc.vector.scalar_tensor_tensor(
                                Y[:, cs], pse[nn], rstd[:],
                                nmr[:, 0:1].to_broadcast((P, TQ)),
                                OP.mult, OP.add,
                            )
                        if use_lnw:
                            nc.vector.tensor_tensor(
                                Y[:, cs], Y[:, cs], lng_rep[:, cs], OP.mult
                            )
                            nc.gpsimd.tensor_tensor(
                                Y[:, cs], Y[:, cs], lnb_rep[:, cs], OP.add
                            )
                        eng = nc.sync if (2 * qc + nn) % 2 == 0 else nc.scalar
                        eng.dma_start(out=dr["y"].ap()[qsl, cs], in_=Y[:, cs])

                return mm, stat, finish

            wo = {}  # qc -> (mm, stat, finish)

            def make_osteps(po0, po1, ets, g0, g1):
                steps = []
                for jc in range(NK):
                    steps.append(
                        lambda jc=jc: nc.tensor.matmul(
                            po0[0 : D + 1, :], VA[jc][:, g0, :],
                            ets[jc][:, 0, :],
                            start=(jc == 0), stop=(jc == NK - 1),
                        )
                    )
                for jc in range(NK):
                    steps.append(
                        lambda jc=jc: nc.tensor.matmul(
                            po1[0 : D + 1, :], VA[jc][:, g1, :],
                            ets[jc][:, 1, :],
                            start=(jc == 0), stop=(jc == NK - 1),
                        )
                    )
                return steps

            def norm(c, po0, po1):
                # d rows live on psum partition 64: copy out, DMA-shift to
                # partition 0 (approx-recip ucode is broken at base!=0),
                # reciprocal, then DRAM-bounce partition broadcast.
                dsb = pb.tile([P, 2 * TQ], F32, tag="dsb")
                dp0 = pb.tile([1, 2 * TQ], F32, tag="dp0")
                rp0 = pb.tile([1, 2 * TQ], F32, tag="rp0")
                rrep = pb.tile([D, 2 * TQ], F32, tag="rrep")
                nc.vector.tensor_scalar_mul(
                    dsb[D : D + 1, 0:TQ], po0[D : D + 1, :], 1.0 / OSCALE
                )
                nc.vector.tensor_scalar_mul(
                    dsb[D : D + 1, TQ:], po1[D : D + 1, :], 1.0 / OSCALE
                )
                nc.sync.dma_start(out=dp0[0:1, :], in_=dsb[D : D + 1, :])
                nc.vector.reciprocal_approx_fast(out=rp0[:], in_=dp0[:])
                rdram = pbd.tile([1, 2 * TQ], F32, tag="rdram")
                nc.sync.dma_start(out=rdram[:], in_=rp0[0:1, :])
                src = rdram[0:1, :]
                bcast = bass.AP(
                    tensor=src.tensor, offset=src.offset, ap=[[0, D]] + src.ap[1:]
                )
                nc.sync.dma_start(out=rrep[:], in_=bcast)
                # normalize: even head straight into OT, odd staged + DMA shift
                odd = pb.tile([D, TQ], OT_T, tag="odd")
                nc.vector.tensor_tensor(
                    OTp[c // 2][0:D, c % 2, :], po0[0:D, :], rrep[:, 0:TQ], OP.mult
                )
                nc.vector.tensor_tensor(
                    odd[:], po1[0:D, :], rrep[:, TQ:], OP.mult
                )
                nc.sync.dma_start(out=OTp[c // 2][D:P, c % 2, :], in_=odd[:])

            QTc, KTc, steps = emit_qk(0)
            for st in steps:
                st()
            prev = None  # (c-1, po0, po1) awaiting normalize
            for c in range(NPAIR):
                h0, h1 = 2 * c, 2 * c + 1
                if c + 1 < NPAIR:
                    nQT, nKT, nsteps = emit_qk(c + 1)
                else:
                    nQT, nKT = None, None
                    # pair 7's extra fill = Wo chain qc0 (freed psP bufs)
                    pw0 = psP.tile([P, TQ], F32, tag="psp", name="wo0a")
                    pw1 = psP.tile([P, TQ], F32, tag="psp", name="wo0b")
                    wo[0] = wo_make(0, [pw0[:], pw1[:]])
                    nsteps = [wo[0][0](nn, mc) for nn in range(2) for mc in (0, 2, 4)]

                # this pair's O psums; its matmuls run as the NEXT pair's
                # fill (they depend only on already-finished exps, so they
                # are always-ready work during the serial exp chain).
                po0 = psO.tile([P, TQ], F32, tag="po0", bufs=2)
                po1 = psO.tile([P, TQ], F32, tag="po1", bufs=2)
                ets = [
                    pb.tile([P, 2, TQ], BF16, tag=f"et{jc}", bufs=2, name=f"et{jc}")
                    for jc in range(NK)
                ]
                osteps = make_osteps(po0, po1, ets, h0, h1)
                if c + 1 < NPAIR:
                    # QK fill first (its results gate the next pair's S);
                    # the previous pair's O steps absorb the late slots.
                    if prev is not None:
                        nsteps = nsteps + prev[3]
                else:
                    if prev is not None:
                        nsteps = prev[3] + nsteps
                    nsteps = nsteps + osteps

                nfill = len(nsteps)
                cuts = [0]
                for jc in range(NK):
                    frac = 0 if jc == 0 else jc / (NK - 1)
                    cuts.append(round(nfill * frac))
                for jc in range(NK):
                    js = slice(jc * P, (jc + 1) * P)
                    s01 = psS.tile([P, 2, TQ], F32, tag="s01", bufs=1)
                    nc.tensor.matmul(
                        s01[:, 0, :],
                        KTc[0:D, js],
                        QTc[0:D, :],
                        start=True, stop=True,
                        tile_position=(0, 0),
                    )
                    nc.tensor.matmul(
                        s01[:, 1, :],
                        KTc[D:P, js],
                        QTc[D:P, :],
                        start=True, stop=True,
                        tile_position=(D, 0),
                    )
                    nc.scalar.activation(
                        out=ets[jc][:], in_=s01[:],
                        func=AF.Exp, scale=0.125,
                    )
                    for st in nsteps[cuts[jc] : cuts[jc + 1]]:
                        st()
                for st in nsteps[cuts[NK] :]:
                    st()

                if prev is not None:
                    norm(prev[0], prev[1], prev[2])
                prev = (c, po0, po1, osteps)
                QTc, KTc = nQT, nKT
            norm(prev[0], prev[1], prev[2])

            # ---- remaining output-projection chains + LN. qc2 takes the
            # s01 bufs (free after pair 7's exps); qc3 reuses qc0's psP
            # bufs once qc0's LN apply has drained them. Finishers are
            # interleaved so the applies overlap later chains' matmuls.
            pw1a = psO.tile([P, TQ], F32, tag="po0", bufs=2, name="wo1a")
            pw1b = psO.tile([P, TQ], F32, tag="po1", bufs=2, name="wo1b")
            wo[1] = wo_make(1, [pw1a[:], pw1b[:]])
            for nn in range(2):
                for mc in (0, 2, 4):
                    wo[1][0](nn, mc)()
            for q in (0, 1):
                for nn in range(2):
                    wo[q][0](nn, 6)()
            sw2 = psS.tile([P, 2, TQ], F32, tag="s01", bufs=1, name="wo2")
            wo[2] = wo_make(2, [sw2[:, 0, :], sw2[:, 1, :]])
            for nn in range(2):
                for mc in (0, 2, 4, 6):
                    wo[2][0](nn, mc)()
            pw3a = psO.tile([P, TQ], F32, tag="po0", bufs=2, name="wo3a")
            pw3b = psO.tile([P, TQ], F32, tag="po1", bufs=2, name="wo3b")
            wo[3] = wo_make(3, [pw3a[:], pw3b[:]])
            for nn in range(2):
                for mc in (0, 2, 4, 6):
                    wo[3][0](nn, mc)()
            # finishers: last accumulation (needs pair 7's OT), stats, LN
            for q in (0, 1, 2, 3):
                mm, stat, finish = wo[q]
                for nn in range(2):
                    mm(nn, 7)()
                    stat(nn)()
                finish()


def _build(NK=NT, use_lnw=True, use_boe=True):
    nc = bacc.Bacc("TRN2", target_bir_lowering=False, debug=False, num_devices=8)
    dr = {}
    dr["xT"] = nc.dram_tensor("xT", [P, NC, NK * P], PROJ_T, kind="ExternalInput")
    dr["xTq"] = nc.dram_tensor("xTq", [P, NC, TQ], PROJ_T, kind="ExternalInput")
    for w in ("Wq", "Wk"):
        dr[w] = nc.dram_tensor(w, [P, NC, C], PROJ_T, kind="ExternalInput")
    dr["Wo"] = nc.dram_tensor("Wo", [P, NC, C], OT_T, kind="ExternalInput")
    dr["Wv"] = nc.dram_tensor("Wv", [P, 2, NC, TQ], PROJ_T, kind="ExternalInput")
    dr["vecs"] = nc.dram_tensor("vecs", [P, NC, 3], F32, kind="ExternalInput")
    dr["maskv"] = nc.dram_tensor("maskv", [P, NK], BF16, kind="ExternalInput")
    if use_boe:
        dr["boe"] = nc.dram_tensor("boe", [1, C], BF16, kind="ExternalInput")
    if use_lnw:
        for v in ("lng", "lnb"):
            dr[v] = nc.dram_tensor(v, [1, C], F32, kind="ExternalInput")
    dr["y"] = nc.dram_tensor("y", [TQ, C], BF16, kind="ExternalOutput")
    with tile.TileContext(nc) as tc:
        _emit(nc, tc, dr, NK, use_lnw, use_boe)
    nc.compile()
    return nc


def _chunk(a):
    """[C, N] -> [128, C//128, N] with [p, c, n] = a[128c+p, n]."""
    return np.ascontiguousarray(
        a.reshape(NC, P, -1).transpose(1, 0, 2)
    )


def _prep_inputs(inputs):
    f32 = np.float32
    Wq = np.asarray(inputs["Wq"], f32)
    Wk = np.asarray(inputs["Wk"], f32)
    Wv = np.asarray(inputs["Wv"], f32)
    Wo = np.asarray(inputs["Wo"], f32)
    x = np.asarray(inputs["x"], f32)
    mask = np.asarray(inputs["attn_mask"]).reshape(B, T)
    # sort keys so unmasked come first; masked tail chunks are dropped
    perms = [np.argsort(-mask[b], kind="stable") for b in range(B)]
    m1max = max(int(mask[b].sum()) for b in range(B))
    NK = min(NT, max(1, -(-m1max // P)))
    KL = NK * P
    bq = np.asarray(inputs["bq"], f32)
    bk = np.asarray(inputs["bk"], f32)
    bv = np.asarray(inputs["bv"], f32)
    bo = np.asarray(inputs["bo"], f32)
    ln_g = np.asarray(inputs["ln_g"], f32)
    ln_b = np.asarray(inputs["ln_b"], f32)

    boe = (bv @ Wo + bo).astype(f32)
    use_boe = bool(np.any(boe != 0.0))
    use_lnw = bool(np.any(ln_g != 1.0) or np.any(ln_b != 0.0))

    def _fp8(a):
        return np.clip(a, -240.0, 240.0).astype(NPFP8)

    def _proj(a):
        return _fp8(a * WSCALE) if QK_FP8 else a.astype(NPBF16)

    def _projx(a):
        return _fp8(a) if QK_FP8 else a.astype(NPBF16)

    shared = {
        "Wq": _proj(_chunk(Wq)),
        "Wk": _proj(_chunk(Wk)),
        "Wv": _proj(
            np.ascontiguousarray(
                _chunk(Wv).reshape(P, NC, 2, TQ).transpose(0, 2, 1, 3)
            )
        ),
        "Wo": _fp8(_chunk(Wo) * WSCALE)
        if WO_FP8
        else _chunk(Wo).astype(NPBF16),
    }
    if use_boe:
        bsc = (WSCALE * OSCALE) if WO_FP8 else 1.0
        shared["boe"] = (boe * bsc).reshape(1, C).astype(NPBF16)
    if use_lnw:
        shared["lng"] = ln_g.reshape(1, C).astype(f32)
        shared["lnb"] = ln_b.reshape(1, C).astype(f32)
    in_maps = []
    for core in range(8):
        b, half = core // 2, core % 2
        xt = np.ascontiguousarray(x[b].T)  # [C, T]
        pk = perms[b][:KL]
        mfp = mask[b][pk].astype(f32)     # permuted/truncated key mask
        vcol = np.zeros((P, NC), f32)
        vcol[:, :NK] = mfp.reshape(NK, P).T / PSCALE
        vecs = np.stack(
            [
                bq.reshape(NC, P).T * PSCALE,
                bk.reshape(NC, P).T * PSCALE,
                vcol,
            ],
            axis=-1,
        )
        m = dict(shared)
        m["xT"] = _projx(_chunk(np.ascontiguousarray(xt[:, pk])))
        m["xTq"] = _projx(_chunk(xt[:, half * TQ : (half + 1) * TQ]))
        m["vecs"] = np.ascontiguousarray(vecs, f32)
        m["maskv"] = np.ascontiguousarray(mfp.reshape(NK, P).T.astype(NPBF16))
        in_maps.append(m)
    return NK, use_lnw, use_boe, in_maps


def kernel(**inputs):
    global LAST_RESULTS
    NK, use_lnw, use_boe, in_maps = _prep_inputs(inputs)
    key = (
        "nc", NK, use_lnw, use_boe,
        os.environ.get("BASS_NWARM", "12"), WO_FP8, QK_FP8,
    )
    if key not in _CACHE:
        _CACHE[key] = _build(NK=NK, use_lnw=use_lnw, use_boe=use_boe)
    nc = _CACHE[key]

    trace = os.environ.get("KERNEL_TRACE", "0") == "1"
    if trace:
        _ensure_ntff_hook()
    LAST_RESULTS = run_bass_kernel_spmd(
        nc, in_maps, core_ids=list(range(8)), trace=trace
    )
    out = np.empty((B, T, C), np.float32)
    for core in range(8):
        b, half = core // 2, core % 2
        out[b, half * TQ : (half + 1) * TQ, :] = np.asarray(
            LAST_RESULTS.results[core]["y"], dtype=np.float32
        )
    return out
